# revision 27
# baseline (speedup 1.0000x reference)
"""Trainium2 Bass kernel: BertCL mean-pool + NT-Xent contrastive loss.

Contract: kernel(last_hidden_states [256,512,768] f32, input_mask [256,512] f32)
-> scalar f32 loss, numerically matching the jax reference.

Strategy (8 NeuronCores, SPMD), "v2" (see _body_v2; v1 kept for reference):
  Batch axis sharded STRIDED: core c owns logical batches {c, c+8, ...}
  (local j <-> logical c + 8j).  Only rows 0..63 ("left" rows) of z enter
  the loss as logsumexp rows / pair terms, and under the strided sharding
  those are exactly locals j<8 of every core.

  stage 1 (memory-bound, ~140.6us/core HBM roofline): per batch, stream
    [512,768] as a [128, 4*768] tile ("(p c) e" layout -> 12KB contiguous
    runs per partition, two half-tile DMAs), fold the 4 sequence chunks on
    the otherwise-idle DVE (PE at cold 1.2GHz was the bottleneck when it
    streamed all 3072 columns), then one-hot-lhsT matmuls accumulate
    8-batch groups into [8,512]+[8,256] PSUM banks -> pooled rows land
    partition-aligned in SBUF.  Mask division cancels in L2-normalization
    and is skipped.
  Hidden mid-stream: after group 0 (locals 0..7 = logical 0..63 across
    cores) an AllGather ships raw sums; each core normalizes the gathered
    [64,768] (1/tau folded into the norm), PE-transposes it, computes
    S_LL = zL zL^T, the pair-sum term, and per own-group S_LO columns
    zL . zhat_own -- all overlapped with the remaining DMA stream.
  Tail (the only exposed part), "fastfin": group 3 is pooled TRANSPOSED
    (per batch, 6 ones-column matmuls write pooledT chunk columns into a
    PSUM bank), so the tail needs one [128,48] PSUM->SBUF copy, norms via
    ones-matmuls, S_OL = zzT3^T zLT directly (no row-normalize pass, no
    PE transposes), exp + ones-matmul -> per-core partial denominators
    D_i^c = sum_{k in own rows} exp(S_ik).  A [64]-float AllGather
    (cheaper than AllReduce) ships partials; the post chain stays in
    [64,1] column form: PE ones-matmul sums the 8 cores, Ln with the
    diagonal removed as a constant per-partition bias (each row is owned
    exactly once), weighted accumulate (triu/cnt pre-scaled by
    -2/n*(n-1)) -> scalar.
  warm_cc=True adds a dummy 4-byte AllGather at t=0 that absorbs the
    cold-start collective setup under the stream on a cold single-shot
    run; it costs ~3us per iteration steady-state, so it is off by
    default.

  Tuning (serialized K-differential, see perf_lab.py/lab.py, 2026-08-10):
  ~176-182us vs ~192us for the previous config and the 140.6us HBM
  roofline.  Key measured facts: the stream must issue from the SINGLE
  sync HWDGE ring - any dual-ring scheme (sync/scalar alternation, mix,
  SWDGE) costs 10-25us in the full kernel despite helping a DMA-only
  stream; xin_bufs=12 (vs 6) saves ~10us of issue stalls; fastfin +
  AllGather-finish cuts the exposed tail from ~33us to ~20us.  Relative
  error vs fp32 jax: 2.7e-7 on HW.

  NOTE: fused DVE ops (tensor_tensor_reduce, scalar_tensor_tensor) pass
  CoreSim but hang/crash this hardware - only plain DVE ops are used.
  PE operand APs must start at partition 0/32/64; matmul accumulation
  start=True clears the whole PSUM bank (PSUM tiles are bank-granular:
  zzT3/S_OL share one bank with transitive cross-engine ordering making
  the bank clears safe).
"""

import sys
from contextlib import ExitStack

import numpy as np

_REPO = "/opt/trn_rl_repo"
if _REPO not in sys.path:
    sys.path.insert(0, _REPO)

import concourse.bass as bass  # noqa: E402  (kept for callers/debugging)
import concourse.tile as tile  # noqa: E402
from concourse import bacc, bass_utils, mybir  # noqa: E402

N_CORES = 8
B, S, H = 256, 512, 768
B_SH = B // N_CORES  # 32 local batches per core
HALF = B_SH // 2  # 16
N_PAIR = B // 4  # 64
TAU = 0.5
F32 = mybir.dt.float32
AX = mybir.AxisListType
AF = mybir.ActivationFunctionType
NEG = -30000.0  # diagonal mask value; exp(NEG + logit) == 0 exactly in fp32


def _body(
    tc,
    x,
    ident,
    dmask,
    triu,
    cnt,
    out,
    use_collective=True,
    stages=("s1", "pool", "cc", "s2"),
    prefix="",
    xlayout="cp",
):
    nc = tc.nc
    P_ = prefix

    with ExitStack() as ctx:
        const = ctx.enter_context(tc.tile_pool(name=f"{P_}const", bufs=1))
        ones_col = const.tile([128, 1], F32)
        nc.vector.memset(ones_col[:], 1.0)
        idt = const.tile([128, 128], F32)
        nc.sync.dma_start(idt[:], ident[:])

        dram = ctx.enter_context(tc.tile_pool(name=f"{P_}dram", bufs=1, space="DRAM"))
        cc_in = dram.tile([B_SH, H], F32)
        shared = "Shared" if use_collective else "Local"
        # asymmetric split: gather locals [0,24) early (hides under the last 8
        # batches' streaming), locals [24,32) at the end (only 64 logical rows
        # of consume work left after the final latency-bound collective)
        SEG = [(0, 16), (16, 32)]
        cc_o = [
            dram.tile([8 * (j1 - j0), H], F32, addr_space=shared, name=f"{P_}cc_o{h}")
            for h, (j0, j1) in enumerate(SEG)
        ]

        # staging row for pooled sums: [1, 32*768] on partition 0
        pooled_sb = const.tile([1, B_SH * H], F32)

        xin = ctx.enter_context(tc.tile_pool(name=f"{P_}xin", bufs=6))
        ps1 = ctx.enter_context(tc.tile_pool(name=f"{P_}ps1", bufs=2, space="PSUM"))
        s2 = ctx.enter_context(tc.tile_pool(name=f"{P_}s2", bufs=1))
        s2t = ctx.enter_context(tc.tile_pool(name=f"{P_}s2t", bufs=2))
        psT = ctx.enter_context(tc.tile_pool(name=f"{P_}psT", bufs=2, space="PSUM"))
        psS = ctx.enter_context(tc.tile_pool(name=f"{P_}psS", bufs=1, space="PSUM"))

        # zT[:, k*256 + p] = z[p, k*128 + q] for partition q (h on partitions)
        zT = s2.tile([128, 6 * B], F32)
        pS = psS.tile([N_PAIR, B], F32)

        def send_half(h):
            """Gather raw sums for local rows [16h,16h+16).

            The reference divides pooled sums by the mask row-sum before
            L2-normalizing; that per-row positive scalar cancels exactly in
            the normalization, so we gather raw sums and normalize the
            gathered rows (same result to ~1ulp, and the pre-collective
            tail shrinks to a single DMA)."""
            j0, j1 = SEG[h]
            nc.sync.dma_start(
                cc_in[j0:j1, :],
                pooled_sb[0:1, j0 * H : j1 * H].rearrange("o (b e) -> o b e", e=H),
            )

            if use_collective:
                nc.gpsimd.collective_compute(
                    "AllGather",
                    mybir.AluOpType.bypass,
                    replica_groups=[list(range(N_CORES))],
                    ins=[cc_in[j0:j1, :].opt()],
                    outs=[cc_o[h].opt()],
                )
            else:
                n = j1 - j0
                for c in range(N_CORES):
                    nc.sync.dma_start(
                        cc_o[h][c * n : (c + 1) * n, :], cc_in[j0:j1, :]
                    )

        def consume_block(h, ja, jb, name):
            """Normalize logical rows [8*ja, 8*jb) from gather h; fill zT cols.

            Gathered row (c, j - SEG[h][0]) holds logical batch c + 8j; the
            permuted 3-D AP (j, c, e) lands partitions in logical order."""
            P = 8 * (jb - ja)  # rows in this block
            col = 8 * ja  # zT column base = first logical row
            zh = s2.tile([P, H], F32, tag=name, name=name)
            src = cc_o[h].rearrange("(c j) e -> j c e", c=N_CORES)
            nc.sync.dma_start(zh[:], src[ja - SEG[h][0] : jb - SEG[h][0]])
            sqs = s2t.tile([P, H], F32, tag=f"sqs{name}", name=f"sqs{name}")
            ssn = s2t.tile([P, 1], F32, tag=f"ssn{name}", name=f"ssn{name}")
            nc.vector.tensor_mul(sqs[:], zh[:], zh[:])
            nc.vector.reduce_sum(out=ssn[:], in_=sqs[:], axis=AX.X)
            # sqrt(TAU * ss): scales z by 1/sqrt(tau) so S = z'z'^T = logits
            nrm = s2t.tile([P, 1], F32, tag=f"nrm{name}", name=f"nrm{name}")
            nc.scalar.activation(nrm[:], ssn[:], AF.Sqrt, scale=TAU)
            rn = s2t.tile([P, 1], F32, tag=f"rn{name}", name=f"rn{name}")
            nc.vector.reciprocal(rn[:], nrm[:])
            nc.vector.tensor_scalar_mul(zh[:], zh[:], rn[:, 0:1])
            for k in range(6):
                pt = psT.tile([128, 128], F32, tag="pt")
                nc.tensor.transpose(
                    pt[:, 0:P], zh[:, k * 128 : (k + 1) * 128], idt[0:P, 0:P]
                )
                nc.vector.tensor_copy(
                    zT[:, k * B + col : k * B + col + P], pt[:, 0:P]
                )

        def logits_block(col, n):
            """S[0:64, col:col+n] += sum_k zT_k[:, 0:64].T @ zT_k[:, col:col+n]"""
            for k in range(6):
                nc.tensor.matmul(
                    pS[:, col : col + n],
                    lhsT=zT[:, k * B : k * B + N_PAIR],
                    rhs=zT[:, k * B + col : k * B + col + n],
                    start=(k == 0),
                    stop=(k == 5),
                )

        # ---- stage 1: per-batch sum over the sequence axis -------------------
        if xlayout == "pc":
            # partition p <- rows 4p..4p+3: contiguous 12KB DMA runs/partition
            x4 = x.rearrange("b (p c) e -> b p c e", c=4)  # [32, 128, 4, 768]
        else:
            # partition p <- rows p, 128+p, ...: 4x 3KB runs/partition
            x4 = x.rearrange("b (c p) e -> b p c e", p=128)  # [32, 128, 4, 768]
        for b in range(B_SH):
            if "s1" in stages:
                xt = xin.tile([128, 4 * H], F32)
                nc.sync.dma_start(xt[:], x4[b])
            if "pool" in stages:
                ps = ps1.tile([1, H], F32)
                for c in range(4):
                    nc.tensor.matmul(
                        ps[:, 0:512],
                        lhsT=ones_col[:, 0:1],
                        rhs=xt[:, c * H : c * H + 512],
                        start=(c == 0),
                        stop=(c == 3),
                    )
                for c in range(4):
                    nc.tensor.matmul(
                        ps[:, 512:H],
                        lhsT=ones_col[:, 0:1],
                        rhs=xt[:, c * H + 512 : (c + 1) * H],
                        start=(c == 0),
                        stop=(c == 3),
                    )
                nc.scalar.copy(pooled_sb[0:1, b * H : (b + 1) * H], ps[:])
            if "cc" in stages:
                if b == SEG[0][1] - 1:
                    send_half(0)
                elif b == SEG[1][1] - 1:
                    send_half(1)

        if "cc" not in stages or "s2" not in stages:
            return
        # each gather carries a full 128-row half of z
        consume_block(0, 0, 16, "zb0")
        logits_block(0, 128)
        consume_block(1, 16, 32, "zb1")
        logits_block(128, 128)

        # ---- finish: masked logsumexp + pair sum ----------------------------
        # pS already holds logits (1/tau folded into the normalization)
        dm = s2.tile([N_PAIR, B], F32)
        nc.sync.dma_start(dm[:], dmask[:])
        sd = s2.tile([N_PAIR, B], F32)
        nc.vector.tensor_add(sd[:], pS[:], dm[:])

        # logits are cosine/tau in [-2,2]: exp is safe without max-subtraction
        et = s2.tile([N_PAIR, B], F32)
        se = s2.tile([N_PAIR, 1], F32)
        nc.scalar.activation(et[:], sd[:], AF.Exp, scale=1.0, accum_out=se[:])
        ld = s2.tile([N_PAIR, 1], F32)
        nc.scalar.activation(ld[:], se[:], AF.Ln)  # logden

        # sum_{i<j<n} (logden[i] - logits[i,j])
        #   = sum_i cnt[i]*logden[i] - sum_ij triu[i,j]*logits[i,j]
        tri_t = s2.tile([N_PAIR, N_PAIR], F32)
        nc.sync.dma_start(tri_t[:], triu[:])
        cnt_t = s2.tile([N_PAIR, 1], F32)
        nc.sync.dma_start(cnt_t[:], cnt[:])
        mt2 = s2.tile([N_PAIR, N_PAIR], F32)
        nc.vector.tensor_mul(mt2[:], sd[0:N_PAIR, 0:N_PAIR], tri_t[:])
        rs = s2.tile([N_PAIR, 1], F32)
        nc.vector.reduce_sum(out=rs[:], in_=mt2[:], axis=AX.X)
        t1 = s2.tile([N_PAIR, 1], F32)
        nc.vector.tensor_mul(t1[:], ld[:], cnt_t[:])
        pr = s2.tile([N_PAIR, 1], F32)
        nc.vector.tensor_sub(pr[:], t1[:], rs[:])

        ptot = psS.tile([1, 1], F32, tag="ptot")
        nc.tensor.matmul(
            ptot[:], lhsT=pr[:], rhs=ones_col[0:N_PAIR, 0:1], start=True, stop=True
        )
        res = s2.tile([1, 1], F32)
        nc.vector.tensor_scalar_mul(res[:], ptot[:], -2.0 / N_PAIR * (N_PAIR - 1))
        nc.sync.dma_start(out[0:1, 0:1], res[:])


def _body_v2(
    tc,
    x,
    ident,
    triu,
    cnt,
    out,
    use_collective=True,
    stages=("s1", "pool", "cc", "s2"),
    prefix="",
    xlayout="pc",
    final_ag=True,  # AllGather + local reduce beats AllReduce (shorter finish)
    stream_split=2,  # DMAs per batch for b < 31 (batch 31 keeps its 2-half path)
    dma_eng="sync",  # "sync" | "alt" (sync/scalar) | "altg" (sync/gpsimd) | "mix"
    cc_gp=False,  # consts + consume DMAs via SWDGE (keeps HWDGE rings clean)
    skip_finish=False,  # drop the final collective + loss chain (timing decomp)
    fin_cut=None,  # None|"pre"|"se"|"dma"|"cc": truncate the finish (decomp)
    warm=True,  # PE warm-up spam before the tail
    fastfin=True,  # group 3 via transposed pooling -> S_OL (shorter tail chain)
    act_light=False,  # move ACT copies to DVE (frees the scalar HWDGE ring)
    xin_bufs=12,  # stream double-buffer depth (12 absorbs consume-phase stalls)
    warm_cc=False,  # tiny AllGather at t=0: absorbs cold-collective setup on a
    # cold single-shot run, but costs ~3us/rep steady-state - off by default
):
    """v2: the only exposed collective is a [64]-float AllReduce.

    Strided sharding (core c owns logical batches c+8j) makes logical rows
    0..63 exactly the locals j<8 of every core, so the "left" rows the loss
    needs are gathered ~25% into the stream (fully hidden).  Each core then
    forms partial NT-Xent denominators D_i^c = sum_{k in own 32 rows}
    exp(zhat_i . zhat_k) locally (its own rows never need to be gathered),
    and the final collective is AllReduce(D) of 64 floats + a ~2us chain.
    The diagonal term exp(zhat_i.zhat_i)=e^2 appears exactly once across
    cores, so it is removed AFTER the AllReduce as a constant Ln bias.
    triu and cnt arrive pre-scaled by -2/n*(n-1) so the loss is a plain
    accumulate at the end (no final matmul / rescale).

    Pooling accumulates 8-batch groups into [8,512]+[8,256] PSUM banks via
    one-hot lhsT columns, so pooled rows land partition-aligned (no
    single-row staging, no DRAM round trip for the own-row path).
    """
    nc = tc.nc
    P_ = prefix
    GRP = 8  # batches per pooling group
    NG = B_SH // GRP  # 4 groups
    # small/consume DMAs: SWDGE keeps them off the HWDGE rings the stream uses
    cdma = nc.gpsimd.dma_start if cc_gp else nc.scalar.dma_start
    # act_light: copies go to DVE so ACT (a HWDGE issuer) stays ~idle
    ccopy = nc.vector.tensor_copy if act_light else nc.scalar.copy

    with ExitStack() as ctx:
        const = ctx.enter_context(tc.tile_pool(name=f"{P_}const", bufs=1))
        idt = const.tile([128, 128], F32)
        cdma(idt[:], ident[:])
        # e8: block j (cols 8j..8j+8) has column j all-ones -> one-hot lhsT
        e8 = const.tile([128, 8 * GRP], F32)
        nc.vector.memset(e8[:], 0.0)
        for j in range(GRP):
            nc.vector.memset(e8[:, 9 * j : 9 * j + 1], 1.0)
        triu_t = const.tile([N_PAIR, N_PAIR], F32)
        cdma(triu_t[:], triu[:])
        cnt_t = const.tile([1, N_PAIR], F32)
        cdma(cnt_t[:], cnt[:])
        negd = const.tile([1, 1], F32)  # -e^{1/tau}: diag correction, post-AR
        nc.vector.memset(negd[:], -float(np.exp(1.0 / TAU)))
        if final_ag or fastfin:
            onesc = const.tile([128, 1], F32)
            nc.vector.memset(onesc[:], 1.0)
        if final_ag:
            negd64 = const.tile([N_PAIR, 1], F32)
            nc.vector.memset(negd64[:], -float(np.exp(1.0 / TAU)))
            cnt_col = const.tile([N_PAIR, 1], F32)
            cdma(cnt_col[:], cnt.rearrange("o b -> b o"))
        if fastfin:
            zeros48 = const.tile([128, 6 * GRP], F32)
            nc.vector.memset(zeros48[:], 0.0)

        # persistent SBUF state (own rows split per group: PE operands must
        # sit at base partition 0)
        zzg = [const.tile([GRP, H], F32, name=f"{P_}zz{g}") for g in range(NG)]
        zLT = const.tile([128, 6 * N_PAIR], F32)  # zhat_L^T chunks
        zzT = const.tile([128, 6 * B_SH], F32)  # zhat_own^T chunks
        sLO = const.tile([N_PAIR, B_SH], F32)  # S_LO columns (SBUF accum)

        dram = ctx.enter_context(tc.tile_pool(name=f"{P_}dram", bufs=1, space="DRAM"))
        shared = "Shared" if use_collective else "Local"
        if warm_cc and use_collective:
            wt = const.tile([1, 1], F32)
            nc.vector.memset(wt[:], 0.0)
            wcc_in = dram.tile([1, 1], F32)
            wcc_out = dram.tile(
                [N_CORES, 1], F32, addr_space="Shared", name=f"{P_}wcc"
            )
            nc.scalar.dma_start(wcc_in[:], wt[:])
            nc.gpsimd.collective_compute(
                "AllGather",
                mybir.AluOpType.bypass,
                replica_groups=[list(range(N_CORES))],
                ins=[wcc_in[:].opt()],
                outs=[wcc_out[:].opt()],
            )
        cc_in = dram.tile([GRP, H], F32)
        ccL = dram.tile([N_PAIR, H], F32, addr_space=shared, name=f"{P_}ccL")
        cc2_in = dram.tile([N_PAIR, 1], F32)
        cc2_shape = [N_CORES * N_PAIR, 1] if final_ag else [N_PAIR, 1]
        cc2_out = dram.tile(cc2_shape, F32, addr_space=shared, name=f"{P_}cc2o")

        xin = ctx.enter_context(tc.tile_pool(name=f"{P_}xin", bufs=xin_bufs))
        psA = ctx.enter_context(tc.tile_pool(name=f"{P_}psA", bufs=2, space="PSUM"))
        # fastfin frees a bank for psZ (group 3 never pools into psA/psB, so
        # bufs=1 only stalls a group boundary by the ~0.1us copy drain)
        psB = ctx.enter_context(
            tc.tile_pool(name=f"{P_}psB", bufs=1 if fastfin else 2, space="PSUM")
        )
        psT = ctx.enter_context(tc.tile_pool(name=f"{P_}psT", bufs=2, space="PSUM"))
        psO = ctx.enter_context(tc.tile_pool(name=f"{P_}psO", bufs=1, space="PSUM"))
        psS = ctx.enter_context(tc.tile_pool(name=f"{P_}psS", bufs=1, space="PSUM"))
        psZ = (
            ctx.enter_context(tc.tile_pool(name=f"{P_}psZ", bufs=1, space="PSUM"))
            if fastfin
            else None
        )
        s2 = ctx.enter_context(tc.tile_pool(name=f"{P_}s2", bufs=1))
        s2t = ctx.enter_context(tc.tile_pool(name=f"{P_}s2t", bufs=2))

        pSLL = psS.tile([N_PAIR, N_PAIR], F32)  # S_LL bank
        # one full psZ bank: cols 0:48 = zzT3 (transposed pooled sums),
        # cols 64:128 = the S_OL matmul output (PSUM tiles are bank-granular)
        zzps = psZ.tile([128, 512], F32, name=f"{P_}zzps") if fastfin else None
        zzT3p = zzps

        def consume_L():
            """Normalize gathered left rows, transpose, S_LL, pair-sum."""
            zL = s2.tile([N_PAIR, H], F32, name=f"{P_}zL")
            # gathered row c*8+j holds logical batch c+8j -> partition 8j+c
            src = ccL.rearrange("(c j) e -> j c e", c=N_CORES)
            cdma(zL[:], src)
            sqs = s2t.tile([N_PAIR, H], F32, tag="sqL")
            nc.vector.tensor_mul(sqs[:], zL[:], zL[:])
            ssn = s2t.tile([N_PAIR, 1], F32, tag="ssL")
            nc.vector.reduce_sum(out=ssn[:], in_=sqs[:], axis=AX.X)
            nrm = s2t.tile([N_PAIR, 1], F32, tag="nrL")
            nc.scalar.activation(nrm[:], ssn[:], AF.Sqrt, scale=TAU)
            rn = s2t.tile([N_PAIR, 1], F32, tag="rnL")
            nc.vector.reciprocal(rn[:], nrm[:])
            nc.vector.tensor_scalar_mul(zL[:], zL[:], rn[:, 0:1])
            for k in range(6):
                pt = psT.tile([128, N_PAIR], F32, tag="pt")
                nc.tensor.transpose(
                    pt[:, 0:N_PAIR],
                    zL[:, k * 128 : (k + 1) * 128],
                    idt[0:N_PAIR, 0:N_PAIR],
                )
                ccopy(zLT[:, k * N_PAIR : (k + 1) * N_PAIR], pt[:, 0:N_PAIR])
            for k in range(6):
                nc.tensor.matmul(
                    pSLL[:],
                    lhsT=zLT[:, k * N_PAIR : (k + 1) * N_PAIR],
                    rhs=zLT[:, k * N_PAIR : (k + 1) * N_PAIR],
                    start=(k == 0),
                    stop=(k == 5),
                )
            mt2 = s2.tile([N_PAIR, N_PAIR], F32, name=f"{P_}mt2")
            nc.vector.tensor_mul(mt2[:], pSLL[:], triu_t[:])
            rs = s2.tile([N_PAIR, 1], F32, name=f"{P_}rs")
            nc.vector.reduce_sum(out=rs[:], in_=mt2[:], axis=AX.X)
            if final_ag:
                return rs  # column finish: no transpose needed
            # transpose to [1,64] so the whole finish chain is single-row
            prT = psT.tile([128, N_PAIR], F32, tag="pt")
            nc.tensor.transpose(prT[0:1, 0:N_PAIR], rs[:], idt[0:N_PAIR, 0:N_PAIR])
            rsT = s2.tile([1, N_PAIR], F32, name=f"{P_}rsT")
            ccopy(rsT[:], prT[0:1, 0:N_PAIR])
            return rsT

        def group_consume(g):
            """Normalize own group rows, transpose, S_LO columns for group g."""
            rows = zzg[g][:]
            sq8 = s2t.tile([GRP, H], F32, tag="sq8")
            nc.vector.tensor_mul(sq8[:], rows, rows)
            sn8 = s2t.tile([GRP, 1], F32, tag="sn8")
            nc.vector.reduce_sum(out=sn8[:], in_=sq8[:], axis=AX.X)
            nr8 = s2t.tile([GRP, 1], F32, tag="nr8")
            nc.scalar.activation(nr8[:], sn8[:], AF.Sqrt, scale=TAU)
            rn8 = s2t.tile([GRP, 1], F32, tag="rn8")
            nc.vector.reciprocal(rn8[:], nr8[:])
            nc.vector.tensor_scalar_mul(rows, rows, rn8[:, 0:1])
            for k in range(6):
                pt = psT.tile([128, N_PAIR], F32, tag="pt")
                nc.tensor.transpose(
                    pt[:, 0:GRP], rows[:, k * 128 : (k + 1) * 128], idt[0:GRP, 0:GRP]
                )
                ccopy(
                    zzT[:, k * B_SH + g * GRP : k * B_SH + (g + 1) * GRP],
                    pt[:, 0:GRP],
                )
            pO = psO.tile([N_PAIR, GRP], F32, tag="pO")
            for k in range(6):
                nc.tensor.matmul(
                    pO[:],
                    lhsT=zLT[:, k * N_PAIR : (k + 1) * N_PAIR],
                    rhs=zzT[:, k * B_SH + g * GRP : k * B_SH + (g + 1) * GRP],
                    start=(k == 0),
                    stop=(k == 5),
                )
            if g < NG - 1:
                ccopy(sLO[:, g * GRP : (g + 1) * GRP], pO[:])
            return pO

        # ---- stage 1: stream + grouped pooling ------------------------------
        if xlayout == "pc":
            x4 = x.rearrange("b (p c) e -> b p c e", c=4)
        else:
            x4 = x.rearrange("b (c p) e -> b p c e", p=128)
        rsT = None
        se_a = [None]
        pA = pB = None
        for b in range(B_SH):
            g, jg = divmod(b, GRP)
            if "s1" in stages:
                # two half-tile DMAs on the last batch so the chunk-folding
                # adds start when the first half lands (shaves ~2us off the
                # tail); earlier batches use stream_split (bigger DMAs have
                # better HBM efficiency)
                # mix: scalar ring only where ACT compute is guaranteed quiet
                # (consume_L/group_consume land at b=15-17 and 27, exp at 27)
                MIXSB = {1, 3, 5, 7, 9, 11, 13, 21, 23, 25, 29}
                if dma_eng == "alt" and b % 2:
                    eng = nc.scalar
                elif dma_eng == "altg" and b % 2:
                    eng = nc.gpsimd
                elif dma_eng == "mix" and b in MIXSB:
                    eng = nc.scalar
                else:
                    eng = nc.sync
                xt = xin.tile([128, 4 * H], F32)
                nsp = 2 if b == B_SH - 1 else stream_split
                w = 4 * H // nsp
                cw = 4 // nsp
                for s in range(nsp):
                    e_s = eng
                    if dma_eng == "mix" and b == B_SH - 1:
                        # last batch: one half per ring for earliest landing
                        e_s = nc.sync if s == 0 else nc.scalar
                    e_s.dma_start(
                        xt[:, s * w : (s + 1) * w], x4[b, :, s * cw : (s + 1) * cw]
                    )
            if "pool" in stages and fastfin and g == NG - 1:
                # group 3: transposed pooling.  Column 8k+jg of the psZ bank
                # gets pooledT chunk k of this batch via a [128x128]-stationary
                # ones-column matmul, so the tail needs no PSUM->SBUF row copy,
                # no normalize-rows pass, and no PE transposes.
                if jg == 0:
                    # clear the bank once (start=True covers all 48 cols)
                    nc.tensor.matmul(
                        zzT3p[:, 0 : 6 * GRP],
                        lhsT=idt[:],
                        rhs=zeros48[:],
                        start=True,
                        stop=False,
                    )
                nc.vector.tensor_add(xt[:, 0:H], xt[:, 0:H], xt[:, H : 2 * H])
                nc.vector.tensor_add(
                    xt[:, 2 * H : 3 * H],
                    xt[:, 2 * H : 3 * H],
                    xt[:, 3 * H : 4 * H],
                )
                nc.vector.tensor_add(xt[:, 0:H], xt[:, 0:H], xt[:, 2 * H : 3 * H])
                for k in range(6):
                    nc.tensor.matmul(
                        zzT3p[:, 8 * k + jg : 8 * k + jg + 1],
                        lhsT=xt[:, k * 128 : (k + 1) * 128],
                        rhs=onesc[:, 0:1],
                        start=False,
                        stop=True,
                    )
            elif "pool" in stages:
                if jg == 0:
                    pA = psA.tile([GRP, 512], F32, tag="A")
                    pB = psB.tile([GRP, 256], F32, tag="B")
                lw = e8[:, GRP * jg : GRP * (jg + 1)]
                if b == B_SH - 1:
                    # last batch: fold only half 1 on DVE; chunks 2 and 3 go
                    # straight into the (warm) PE accumulation.  Shorter tail
                    # path than add2 -> add3 -> matmul, and drops the DVE->PE
                    # handoff from the critical path.  Emission order: chunks
                    # 2,3 first (only need the half-2 DMA), folded chunk last.
                    nc.vector.tensor_add(xt[:, 0:H], xt[:, 0:H], xt[:, H : 2 * H])
                    for base in (2 * H, 3 * H, 0):
                        nc.tensor.matmul(
                            pA[:, :],
                            lhsT=lw,
                            rhs=xt[:, base : base + 512],
                            start=False,
                            stop=(base == 0),
                        )
                    for base in (2 * H, 3 * H, 0):
                        nc.tensor.matmul(
                            pB[:, :],
                            lhsT=lw,
                            rhs=xt[:, base + 512 : base + H],
                            start=False,
                            stop=(base == 0),
                        )
                else:
                    # fold the 4 sequence chunks on the (otherwise idle) DVE
                    # so PE streams 768 cols/batch instead of 3072 (PE at
                    # cold 1.2GHz was the stream bottleneck at 8 mm/batch)
                    nc.vector.tensor_add(xt[:, 0:H], xt[:, 0:H], xt[:, H : 2 * H])
                    nc.vector.tensor_add(
                        xt[:, 2 * H : 3 * H],
                        xt[:, 2 * H : 3 * H],
                        xt[:, 3 * H : 4 * H],
                    )
                    nc.vector.tensor_add(xt[:, 0:H], xt[:, 0:H], xt[:, 2 * H : 3 * H])
                    nc.tensor.matmul(
                        pA[:, :],
                        lhsT=lw,
                        rhs=xt[:, 0:512],
                        start=(jg == 0),
                        stop=(jg == GRP - 1),
                    )
                    nc.tensor.matmul(
                        pB[:, :],
                        lhsT=lw,
                        rhs=xt[:, 512:H],
                        start=(jg == 0),
                        stop=(jg == GRP - 1),
                    )
                if jg == GRP - 1:
                    # split across ACT and DVE so the two bank copies run in
                    # parallel (group 3's copies sit on the exposed tail);
                    # act_light puts both on DVE to keep the ACT ring clean
                    if act_light:
                        nc.vector.tensor_copy(zzg[g][:, 0:512], pA[:])
                    else:
                        nc.scalar.copy(zzg[g][:, 0:512], pA[:])
                    nc.vector.tensor_copy(zzg[g][:, 512:H], pB[:])
            if "cc" in stages:
                if b == GRP - 1:
                    cdma(cc_in[:], zzg[0][:])
                    if use_collective:
                        nc.gpsimd.collective_compute(
                            "AllGather",
                            mybir.AluOpType.bypass,
                            replica_groups=[list(range(N_CORES))],
                            ins=[cc_in[:].opt()],
                            outs=[ccL[:].opt()],
                        )
                    else:
                        for c in range(N_CORES):
                            nc.scalar.dma_start(
                                ccL[c * GRP : (c + 1) * GRP, :], cc_in[:]
                            )
                if "s2" in stages:
                    if b == 15:
                        rsT = consume_L()
                    elif b == 16:
                        group_consume(0)
                    elif b == 17:
                        group_consume(1)
                    elif b == 27:
                        group_consume(2)
                        # exp for groups 0..2 hidden under the stream
                        eta = s2.tile([N_PAIR, 3 * GRP], F32, name=f"{P_}eta")
                        sa = s2.tile([N_PAIR, 1], F32, name=f"{P_}sea")
                        nc.scalar.activation(
                            eta[:],
                            sLO[:, 0 : 3 * GRP],
                            AF.Exp,
                            scale=1.0,
                            accum_out=sa[:],
                        )
                        se_a[0] = sa
            if "pool" in stages and "s2" in stages and warm and b == B_SH - 2:
                # ~3.5us of back-to-back dummy PE work, hidden under batch
                # 31's DMA window: trips the HAM activity monitor (4096-cycle
                # window) so the tail's matmuls/transposes run at the warm
                # 2.4GHz clock instead of the cold 1.2GHz default
                for _ in range(30):
                    ptw = psT.tile([128, N_PAIR], F32, tag="pt")
                    nc.tensor.transpose(
                        ptw[:, 0:N_PAIR],
                        idt[0:N_PAIR, :],
                        idt[0:N_PAIR, 0:N_PAIR],
                    )

        if "cc" not in stages or "s2" not in stages:
            return
        if skip_finish:
            fin_cut = "pre"
        if fastfin:
            if fin_cut == "pre":
                return
            # ---- fastfin tail: zzT3 (PSUM, transposed raw sums) -> S_OL ----
            # one [128,48] copy replaces the row copy + normalize-rows +
            # 6 transposes of group_consume; norms via ones-column matmuls.
            zzT3sb = s2.tile([128, 6 * GRP], F32, name=f"{P_}zzT3sb")
            nc.scalar.copy(zzT3sb[:], zzT3p[:, 0 : 6 * GRP])
            # S_OL[j, i] = zraw_own_j . zhat_L_i (normalize by rn8f after);
            # emitted FIRST so PE's long pole starts as soon as the copy
            # lands.  Output shares the psZ bank; its start=True clear of
            # zzT3 is safe (zzT3's only reader, the zzT3sb copy, is upstream
            # of these matmuls).
            pOL = zzps[0:GRP, 64 : 64 + N_PAIR]
            for k in range(6):
                nc.tensor.matmul(
                    pOL,
                    lhsT=zzT3sb[:, 8 * k : 8 * (k + 1)],
                    rhs=zLT[:, k * N_PAIR : (k + 1) * N_PAIR],
                    start=(k == 0),
                    stop=(k == 5),
                )
            sq48 = s2t.tile([128, 6 * GRP], F32, tag="sq48")
            nc.vector.tensor_mul(sq48[:], zzT3sb[:], zzT3sb[:])
            pn = psT.tile([128, N_PAIR], F32, tag="pt")
            for k in range(6):
                nc.tensor.matmul(
                    pn[0:GRP, 0:1],
                    lhsT=sq48[:, 8 * k : 8 * (k + 1)],
                    rhs=onesc[:, 0:1],
                    start=(k == 0),
                    stop=(k == 5),
                )
            nr8f = s2t.tile([GRP, 1], F32, tag="nr8f")
            nc.scalar.activation(nr8f[:], pn[0:GRP, 0:1], AF.Sqrt, scale=TAU)
            rn8f = s2t.tile([GRP, 1], F32, tag="rn8f")
            nc.vector.reciprocal(rn8f[:], nr8f[:])
            sOL = s2.tile([GRP, N_PAIR], F32, name=f"{P_}sOL")
            nc.vector.tensor_scalar_mul(sOL[:], pOL, rn8f[:, 0:1])
            eOL = s2.tile([GRP, N_PAIR], F32, name=f"{P_}eOL")
            nc.scalar.activation(eOL[:], sOL[:], AF.Exp, scale=1.0)
            pd3 = psT.tile([128, N_PAIR], F32, tag="pt")
            nc.tensor.matmul(
                pd3[0:N_PAIR, 0:1],
                lhsT=eOL[:],
                rhs=onesc[0:GRP, 0:1],
                start=True,
                stop=True,
            )
            se = s2.tile([N_PAIR, 1], F32, name=f"{P_}se")
            nc.vector.tensor_add(se[:], se_a[0], pd3[0:N_PAIR, 0:1])
        else:
            pO3 = group_consume(3)
            if fin_cut == "pre":
                return

            # ---- finish: partial denominators -> AllReduce -> loss ----------
            # exp over group-3 columns straight from PSUM; groups 0..2 were
            # exp'd mid-stream (exp_a).  se = se_a + se_b, diag removed later.
            et = s2.tile([N_PAIR, GRP], F32, name=f"{P_}et")
            se_b = s2.tile([N_PAIR, 1], F32, name=f"{P_}seb")
            nc.scalar.activation(et[:], pO3[:], AF.Exp, scale=1.0, accum_out=se_b[:])
            se = s2.tile([N_PAIR, 1], F32, name=f"{P_}se")
            nc.vector.tensor_add(se[:], se_a[0], se_b[:])
        if fin_cut == "se":
            return
        nc.scalar.dma_start(cc2_in[:], se[:])
        if fin_cut == "dma":
            return
        if use_collective:
            nc.gpsimd.collective_compute(
                "AllGather" if final_ag else "AllReduce",
                mybir.AluOpType.bypass if final_ag else mybir.AluOpType.add,
                replica_groups=[list(range(N_CORES))],
                ins=[cc2_in[:].opt()],
                outs=[cc2_out[:].opt()],
            )
        elif final_ag:
            for c in range(N_CORES):
                nc.scalar.dma_start(
                    cc2_out[c * N_PAIR : (c + 1) * N_PAIR, :], cc2_in[:]
                )
        else:
            nc.scalar.dma_start(cc2_out[:], cc2_in[:])
        if fin_cut == "cc":
            return
        if final_ag:
            # column finish: partition c <- core c's [64] (8 contiguous
            # descriptors), PE ones-matmul sums the cores, then the whole
            # chain stays [64,1] (no transposes; rsT here is the rs column).
            l8 = s2.tile([N_CORES, N_PAIR], F32, name=f"{P_}l8")
            nc.scalar.dma_start(
                l8[:], cc2_out[:].rearrange("(c i) o -> c (i o)", c=N_CORES)
            )
            pD = psT.tile([128, N_PAIR], F32, tag="pt")
            nc.tensor.matmul(
                pD[0:N_PAIR, 0:1],
                lhsT=l8[:],
                rhs=onesc[0:N_CORES, 0:1],
                start=True,
                stop=True,
            )
            ldc = s2.tile([N_PAIR, 1], F32, name=f"{P_}ldc")
            nc.scalar.activation(ldc[:], pD[0:N_PAIR, 0:1], AF.Ln, bias=negd64[:])
            t1c = s2.tile([N_PAIR, 1], F32, name=f"{P_}t1c")
            nc.vector.tensor_mul(t1c[:], ldc[:], cnt_col[:])
            prc = s2.tile([N_PAIR, 1], F32, name=f"{P_}prc")
            nc.vector.tensor_sub(prc[:], t1c[:], rsT[:])
            pres = psT.tile([128, N_PAIR], F32, tag="pt")
            nc.tensor.matmul(
                pres[0:1, 0:1],
                lhsT=prc[:],
                rhs=onesc[0:N_PAIR, 0:1],
                start=True,
                stop=True,
            )
            res = s2.tile([1, 1], F32, name=f"{P_}res")
            nc.scalar.copy(res[:], pres[0:1, 0:1])
            nc.sync.dma_start(out[0:1, 0:1], res[:])
            return
        # single-row finish: read D back as [1,64], remove the diagonal
        # (each row owned by exactly one core -> sum of e^{1/tau} once)
        # as a constant Ln bias, then weighted-accumulate to the scalar.
        lds = s2.tile([1, N_PAIR], F32, name=f"{P_}lds")
        nc.scalar.dma_start(lds[:], cc2_out[:].rearrange("a b -> b a"))
        ld = s2.tile([1, N_PAIR], F32, name=f"{P_}ld")
        nc.scalar.activation(ld[:], lds[:], AF.Ln, bias=negd[:])
        t1 = s2.tile([1, N_PAIR], F32, name=f"{P_}t1")
        nc.vector.tensor_mul(t1[:], ld[:], cnt_t[:])
        pr = s2.tile([1, N_PAIR], F32, name=f"{P_}pr")
        nc.vector.tensor_sub(pr[:], t1[:], rsT[:])
        junk = s2.tile([1, N_PAIR], F32, name=f"{P_}junk")
        res = s2.tile([1, 1], F32, name=f"{P_}res")
        nc.scalar.activation(junk[:], pr[:], AF.Copy, accum_out=res[:])
        nc.scalar.dma_start(out[0:1, 0:1], res[:])


KERNEL_VERSION = "v2"


def build_nc(reps=1, version=None, serialize_reps=False, **body_kwargs):
    version = version or KERNEL_VERSION
    nc = bacc.Bacc("TRN2", target_bir_lowering=False, debug=False, num_devices=N_CORES)
    x = nc.dram_tensor("x", [B_SH, S, H], F32, kind="ExternalInput")
    ident = nc.dram_tensor("ident", [128, 128], F32, kind="ExternalInput")
    triu = nc.dram_tensor("triu", [N_PAIR, N_PAIR], F32, kind="ExternalInput")
    if version == "v1":
        cnt = nc.dram_tensor("cnt", [N_PAIR, 1], F32, kind="ExternalInput")
        dmask = nc.dram_tensor("dmask", [N_PAIR, B], F32, kind="ExternalInput")
    else:
        cnt = nc.dram_tensor("cnt", [1, N_PAIR], F32, kind="ExternalInput")
    out = nc.dram_tensor("loss", [1, 1], F32, kind="ExternalOutput")
    with tile.TileContext(nc) as tc:
        for r in range(reps):
            prefix = f"r{r}_" if reps > 1 else ""
            if serialize_reps and r > 0:
                # block this rep's stream-issue rings on the previous rep's
                # final out-write so K-diff measures honest serial per-rep
                # time (no cross-rep overlap games)
                with tc.tile_pool(name=f"ser{r}", bufs=1) as serp:
                    tok = serp.tile([1, 2], F32, name=f"tok{r}")
                    nc.sync.dma_start(tok[0:1, 0:1], out.ap()[0:1, 0:1])
                    nc.scalar.dma_start(tok[0:1, 1:2], out.ap()[0:1, 0:1])
            if version == "v1":
                _body(
                    tc,
                    x.ap(),
                    ident.ap(),
                    dmask.ap(),
                    triu.ap(),
                    cnt.ap(),
                    out.ap(),
                    prefix=prefix,
                    **body_kwargs,
                )
            else:
                _body_v2(
                    tc,
                    x.ap(),
                    ident.ap(),
                    triu.ap(),
                    cnt.ap(),
                    out.ap(),
                    prefix=prefix,
                    **body_kwargs,
                )
    nc.compile()
    return nc


def const_inputs(version=None):
    version = version or KERNEL_VERSION
    ident = np.eye(128, dtype=np.float32)
    triu = np.triu(np.ones((N_PAIR, N_PAIR), dtype=np.float32), k=1)
    cnt = (N_PAIR - 1 - np.arange(N_PAIR, dtype=np.float32)).reshape(N_PAIR, 1)
    if version == "v1":
        dmask = np.zeros((N_PAIR, B), dtype=np.float32)
        dmask[np.arange(N_PAIR), np.arange(N_PAIR)] = NEG
        return {"ident": ident, "triu": triu, "cnt": cnt, "dmask": dmask}
    # v2: fold the final -2/n*(n-1) scale into triu and cnt so the loss is
    # a plain accumulate after the AllReduce
    sc = -2.0 / N_PAIR * (N_PAIR - 1)
    return {
        "ident": ident,
        "triu": (sc * triu).astype(np.float32),
        "cnt": (sc * cnt).astype(np.float32).reshape(1, N_PAIR),
    }


def make_in_maps(last_hidden_states, input_mask, version=None):
    version = version or KERNEL_VERSION
    del input_mask  # cancels exactly in the L2 normalization (see half_tail)
    x = np.asarray(last_hidden_states, dtype=np.float32)
    consts = const_inputs(version)
    return [
        {"x": np.ascontiguousarray(x[c::N_CORES]), **consts}  # logical c+8j
        for c in range(N_CORES)
    ]


_CACHE = {}


def get_nc(version=None):
    key = version or KERNEL_VERSION
    if key not in _CACHE:
        _CACHE[key] = build_nc(version=key)
    return _CACHE[key]


def kernel(last_hidden_states, input_mask):
    nc = get_nc()
    in_maps = make_in_maps(last_hidden_states, input_mask)
    res = bass_utils.run_bass_kernel_spmd(nc, in_maps, core_ids=list(range(N_CORES)))
    return np.asarray(res.results[0]["loss"], dtype=np.float32).reshape(())



# revision 28
# speedup vs baseline: 1.1531x; 1.1531x over previous
"""Trainium2 Bass kernel: BertCL mean-pool + NT-Xent contrastive loss.

Contract: kernel(last_hidden_states [256,512,768] f32, input_mask [256,512] f32)
-> scalar f32 loss, numerically matching the jax reference.

Strategy (8 NeuronCores, SPMD), "v2" (see _body_v2; v1 kept for reference):
  Batch axis sharded STRIDED: core c owns logical batches {c, c+8, ...}
  (local j <-> logical c + 8j).  Only rows 0..63 ("left" rows) of z enter
  the loss as logsumexp rows / pair terms, and under the strided sharding
  those are exactly locals j<8 of every core.

  stage 1 (memory-bound, ~140.6us/core HBM roofline): per batch, stream
    [512,768] as a [128, 4*768] tile ("(p c) e" layout -> 12KB contiguous
    runs per partition, two half-tile DMAs), fold the 4 sequence chunks on
    the otherwise-idle DVE (PE at cold 1.2GHz was the bottleneck when it
    streamed all 3072 columns), then one-hot-lhsT matmuls accumulate
    8-batch groups into [8,512]+[8,256] PSUM banks -> pooled rows land
    partition-aligned in SBUF.  Mask division cancels in L2-normalization
    and is skipped.
  Hidden mid-stream: after group 0 (locals 0..7 = logical 0..63 across
    cores) an AllGather ships raw sums; each core normalizes the gathered
    [64,768] (1/tau folded into the norm), PE-transposes it, computes
    S_LL = zL zL^T, the pair-sum term, and per own-group S_LO columns
    zL . zhat_own -- all overlapped with the remaining DMA stream.
  Tail (the only exposed part), "fastfin": group 3 is pooled TRANSPOSED
    (per batch, 6 ones-column matmuls write pooledT chunk columns into a
    PSUM bank), so the tail needs one [128,48] PSUM->SBUF copy, norms via
    ones-matmuls, S_OL = zzT3^T zLT directly (no row-normalize pass, no
    PE transposes), exp + ones-matmul -> per-core partial denominators
    D_i^c = sum_{k in own rows} exp(S_ik).  A [64]-float AllGather
    (cheaper than AllReduce) ships partials; the post chain stays in
    [64,1] column form: PE ones-matmul sums the 8 cores, Ln with the
    diagonal removed as a constant per-partition bias (each row is owned
    exactly once), weighted accumulate (triu/cnt pre-scaled by
    -2/n*(n-1)) -> scalar.
  warm_cc=True adds a dummy 4-byte AllGather at t=0 that absorbs the
    cold-start collective setup under the stream on a cold single-shot
    run; it costs ~3us per iteration steady-state, so it is off by
    default.

  Tuning (serialized K-differential, see perf_lab.py/lab.py, 2026-08-10):
  ~176-182us vs ~192us for the previous config and the 140.6us HBM
  roofline.  Key measured facts: the stream must issue from the SINGLE
  sync HWDGE ring - any dual-ring scheme (sync/scalar alternation, mix,
  SWDGE) costs 10-25us in the full kernel despite helping a DMA-only
  stream; xin_bufs=12 (vs 6) saves ~10us of issue stalls; fastfin +
  AllGather-finish cuts the exposed tail from ~33us to ~20us.  Relative
  error vs fp32 jax: 2.7e-7 on HW.

  NOTE: fused DVE ops (tensor_tensor_reduce, scalar_tensor_tensor) pass
  CoreSim but hang/crash this hardware - only plain DVE ops are used.
  PE operand APs must start at partition 0/32/64; matmul accumulation
  start=True clears the whole PSUM bank (PSUM tiles are bank-granular:
  zzT3/S_OL share one bank with transitive cross-engine ordering making
  the bank clears safe).
"""

import sys
from contextlib import ExitStack

import numpy as np

_REPO = "/opt/trn_rl_repo"
if _REPO not in sys.path:
    sys.path.insert(0, _REPO)

import concourse.bass as bass  # noqa: E402  (kept for callers/debugging)
import concourse.tile as tile  # noqa: E402
from concourse import bacc, bass_utils, mybir  # noqa: E402

N_CORES = 8
B, S, H = 256, 512, 768
B_SH = B // N_CORES  # 32 local batches per core
HALF = B_SH // 2  # 16
N_PAIR = B // 4  # 64
TAU = 0.5
F32 = mybir.dt.float32
AX = mybir.AxisListType
AF = mybir.ActivationFunctionType
NEG = -30000.0  # diagonal mask value; exp(NEG + logit) == 0 exactly in fp32


def _body(
    tc,
    x,
    ident,
    dmask,
    triu,
    cnt,
    out,
    use_collective=True,
    stages=("s1", "pool", "cc", "s2"),
    prefix="",
    xlayout="cp",
):
    nc = tc.nc
    P_ = prefix

    with ExitStack() as ctx:
        const = ctx.enter_context(tc.tile_pool(name=f"{P_}const", bufs=1))
        ones_col = const.tile([128, 1], F32)
        nc.vector.memset(ones_col[:], 1.0)
        idt = const.tile([128, 128], F32)
        nc.sync.dma_start(idt[:], ident[:])

        dram = ctx.enter_context(tc.tile_pool(name=f"{P_}dram", bufs=1, space="DRAM"))
        cc_in = dram.tile([B_SH, H], F32)
        shared = "Shared" if use_collective else "Local"
        # asymmetric split: gather locals [0,24) early (hides under the last 8
        # batches' streaming), locals [24,32) at the end (only 64 logical rows
        # of consume work left after the final latency-bound collective)
        SEG = [(0, 16), (16, 32)]
        cc_o = [
            dram.tile([8 * (j1 - j0), H], F32, addr_space=shared, name=f"{P_}cc_o{h}")
            for h, (j0, j1) in enumerate(SEG)
        ]

        # staging row for pooled sums: [1, 32*768] on partition 0
        pooled_sb = const.tile([1, B_SH * H], F32)

        xin = ctx.enter_context(tc.tile_pool(name=f"{P_}xin", bufs=6))
        ps1 = ctx.enter_context(tc.tile_pool(name=f"{P_}ps1", bufs=2, space="PSUM"))
        s2 = ctx.enter_context(tc.tile_pool(name=f"{P_}s2", bufs=1))
        s2t = ctx.enter_context(tc.tile_pool(name=f"{P_}s2t", bufs=2))
        psT = ctx.enter_context(tc.tile_pool(name=f"{P_}psT", bufs=2, space="PSUM"))
        psS = ctx.enter_context(tc.tile_pool(name=f"{P_}psS", bufs=1, space="PSUM"))

        # zT[:, k*256 + p] = z[p, k*128 + q] for partition q (h on partitions)
        zT = s2.tile([128, 6 * B], F32)
        pS = psS.tile([N_PAIR, B], F32)

        def send_half(h):
            """Gather raw sums for local rows [16h,16h+16).

            The reference divides pooled sums by the mask row-sum before
            L2-normalizing; that per-row positive scalar cancels exactly in
            the normalization, so we gather raw sums and normalize the
            gathered rows (same result to ~1ulp, and the pre-collective
            tail shrinks to a single DMA)."""
            j0, j1 = SEG[h]
            nc.sync.dma_start(
                cc_in[j0:j1, :],
                pooled_sb[0:1, j0 * H : j1 * H].rearrange("o (b e) -> o b e", e=H),
            )

            if use_collective:
                nc.gpsimd.collective_compute(
                    "AllGather",
                    mybir.AluOpType.bypass,
                    replica_groups=[list(range(N_CORES))],
                    ins=[cc_in[j0:j1, :].opt()],
                    outs=[cc_o[h].opt()],
                )
            else:
                n = j1 - j0
                for c in range(N_CORES):
                    nc.sync.dma_start(
                        cc_o[h][c * n : (c + 1) * n, :], cc_in[j0:j1, :]
                    )

        def consume_block(h, ja, jb, name):
            """Normalize logical rows [8*ja, 8*jb) from gather h; fill zT cols.

            Gathered row (c, j - SEG[h][0]) holds logical batch c + 8j; the
            permuted 3-D AP (j, c, e) lands partitions in logical order."""
            P = 8 * (jb - ja)  # rows in this block
            col = 8 * ja  # zT column base = first logical row
            zh = s2.tile([P, H], F32, tag=name, name=name)
            src = cc_o[h].rearrange("(c j) e -> j c e", c=N_CORES)
            nc.sync.dma_start(zh[:], src[ja - SEG[h][0] : jb - SEG[h][0]])
            sqs = s2t.tile([P, H], F32, tag=f"sqs{name}", name=f"sqs{name}")
            ssn = s2t.tile([P, 1], F32, tag=f"ssn{name}", name=f"ssn{name}")
            nc.vector.tensor_mul(sqs[:], zh[:], zh[:])
            nc.vector.reduce_sum(out=ssn[:], in_=sqs[:], axis=AX.X)
            # sqrt(TAU * ss): scales z by 1/sqrt(tau) so S = z'z'^T = logits
            nrm = s2t.tile([P, 1], F32, tag=f"nrm{name}", name=f"nrm{name}")
            nc.scalar.activation(nrm[:], ssn[:], AF.Sqrt, scale=TAU)
            rn = s2t.tile([P, 1], F32, tag=f"rn{name}", name=f"rn{name}")
            nc.vector.reciprocal(rn[:], nrm[:])
            nc.vector.tensor_scalar_mul(zh[:], zh[:], rn[:, 0:1])
            for k in range(6):
                pt = psT.tile([128, 128], F32, tag="pt")
                nc.tensor.transpose(
                    pt[:, 0:P], zh[:, k * 128 : (k + 1) * 128], idt[0:P, 0:P]
                )
                nc.vector.tensor_copy(
                    zT[:, k * B + col : k * B + col + P], pt[:, 0:P]
                )

        def logits_block(col, n):
            """S[0:64, col:col+n] += sum_k zT_k[:, 0:64].T @ zT_k[:, col:col+n]"""
            for k in range(6):
                nc.tensor.matmul(
                    pS[:, col : col + n],
                    lhsT=zT[:, k * B : k * B + N_PAIR],
                    rhs=zT[:, k * B + col : k * B + col + n],
                    start=(k == 0),
                    stop=(k == 5),
                )

        # ---- stage 1: per-batch sum over the sequence axis -------------------
        if xlayout == "pc":
            # partition p <- rows 4p..4p+3: contiguous 12KB DMA runs/partition
            x4 = x.rearrange("b (p c) e -> b p c e", c=4)  # [32, 128, 4, 768]
        else:
            # partition p <- rows p, 128+p, ...: 4x 3KB runs/partition
            x4 = x.rearrange("b (c p) e -> b p c e", p=128)  # [32, 128, 4, 768]
        for b in range(B_SH):
            if "s1" in stages:
                xt = xin.tile([128, 4 * H], F32)
                nc.sync.dma_start(xt[:], x4[b])
            if "pool" in stages:
                ps = ps1.tile([1, H], F32)
                for c in range(4):
                    nc.tensor.matmul(
                        ps[:, 0:512],
                        lhsT=ones_col[:, 0:1],
                        rhs=xt[:, c * H : c * H + 512],
                        start=(c == 0),
                        stop=(c == 3),
                    )
                for c in range(4):
                    nc.tensor.matmul(
                        ps[:, 512:H],
                        lhsT=ones_col[:, 0:1],
                        rhs=xt[:, c * H + 512 : (c + 1) * H],
                        start=(c == 0),
                        stop=(c == 3),
                    )
                nc.scalar.copy(pooled_sb[0:1, b * H : (b + 1) * H], ps[:])
            if "cc" in stages:
                if b == SEG[0][1] - 1:
                    send_half(0)
                elif b == SEG[1][1] - 1:
                    send_half(1)

        if "cc" not in stages or "s2" not in stages:
            return
        # each gather carries a full 128-row half of z
        consume_block(0, 0, 16, "zb0")
        logits_block(0, 128)
        consume_block(1, 16, 32, "zb1")
        logits_block(128, 128)

        # ---- finish: masked logsumexp + pair sum ----------------------------
        # pS already holds logits (1/tau folded into the normalization)
        dm = s2.tile([N_PAIR, B], F32)
        nc.sync.dma_start(dm[:], dmask[:])
        sd = s2.tile([N_PAIR, B], F32)
        nc.vector.tensor_add(sd[:], pS[:], dm[:])

        # logits are cosine/tau in [-2,2]: exp is safe without max-subtraction
        et = s2.tile([N_PAIR, B], F32)
        se = s2.tile([N_PAIR, 1], F32)
        nc.scalar.activation(et[:], sd[:], AF.Exp, scale=1.0, accum_out=se[:])
        ld = s2.tile([N_PAIR, 1], F32)
        nc.scalar.activation(ld[:], se[:], AF.Ln)  # logden

        # sum_{i<j<n} (logden[i] - logits[i,j])
        #   = sum_i cnt[i]*logden[i] - sum_ij triu[i,j]*logits[i,j]
        tri_t = s2.tile([N_PAIR, N_PAIR], F32)
        nc.sync.dma_start(tri_t[:], triu[:])
        cnt_t = s2.tile([N_PAIR, 1], F32)
        nc.sync.dma_start(cnt_t[:], cnt[:])
        mt2 = s2.tile([N_PAIR, N_PAIR], F32)
        nc.vector.tensor_mul(mt2[:], sd[0:N_PAIR, 0:N_PAIR], tri_t[:])
        rs = s2.tile([N_PAIR, 1], F32)
        nc.vector.reduce_sum(out=rs[:], in_=mt2[:], axis=AX.X)
        t1 = s2.tile([N_PAIR, 1], F32)
        nc.vector.tensor_mul(t1[:], ld[:], cnt_t[:])
        pr = s2.tile([N_PAIR, 1], F32)
        nc.vector.tensor_sub(pr[:], t1[:], rs[:])

        ptot = psS.tile([1, 1], F32, tag="ptot")
        nc.tensor.matmul(
            ptot[:], lhsT=pr[:], rhs=ones_col[0:N_PAIR, 0:1], start=True, stop=True
        )
        res = s2.tile([1, 1], F32)
        nc.vector.tensor_scalar_mul(res[:], ptot[:], -2.0 / N_PAIR * (N_PAIR - 1))
        nc.sync.dma_start(out[0:1, 0:1], res[:])


def _body_v2(
    tc,
    x,
    ident,
    triu,
    cnt,
    out,
    use_collective=True,
    stages=("s1", "pool", "cc", "s2"),
    prefix="",
    xlayout="pc",
    final_ag=True,  # AllGather + local reduce beats AllReduce (shorter finish)
    stream_split=2,  # DMAs per batch for b < 31 (batch 31 keeps its 2-half path)
    dma_eng="sync",  # "sync" | "alt" (sync/scalar) | "altg" (sync/gpsimd) | "mix"
    cc_gp=False,  # consts + consume DMAs via SWDGE (keeps HWDGE rings clean)
    skip_finish=False,  # drop the final collective + loss chain (timing decomp)
    fin_cut=None,  # None|"pre"|"se"|"dma"|"cc": truncate the finish (decomp)
    warm=True,  # PE warm-up spam before the tail
    fastfin=True,  # group 3 via transposed pooling -> S_OL (shorter tail chain)
    act_light=False,  # move ACT copies to DVE (frees the scalar HWDGE ring)
    xin_bufs=12,  # stream double-buffer depth (12 absorbs consume-phase stalls)
    warm_cc=False,  # tiny AllGather at t=0: absorbs cold-collective setup on a
    # cold single-shot run, but costs ~3us/rep steady-state - off by default
):
    """v2: the only exposed collective is a [64]-float AllReduce.

    Strided sharding (core c owns logical batches c+8j) makes logical rows
    0..63 exactly the locals j<8 of every core, so the "left" rows the loss
    needs are gathered ~25% into the stream (fully hidden).  Each core then
    forms partial NT-Xent denominators D_i^c = sum_{k in own 32 rows}
    exp(zhat_i . zhat_k) locally (its own rows never need to be gathered),
    and the final collective is AllReduce(D) of 64 floats + a ~2us chain.
    The diagonal term exp(zhat_i.zhat_i)=e^2 appears exactly once across
    cores, so it is removed AFTER the AllReduce as a constant Ln bias.
    triu and cnt arrive pre-scaled by -2/n*(n-1) so the loss is a plain
    accumulate at the end (no final matmul / rescale).

    Pooling accumulates 8-batch groups into [8,512]+[8,256] PSUM banks via
    one-hot lhsT columns, so pooled rows land partition-aligned (no
    single-row staging, no DRAM round trip for the own-row path).
    """
    nc = tc.nc
    P_ = prefix
    GRP = 8  # batches per pooling group
    NG = B_SH // GRP  # 4 groups
    # small/consume DMAs: SWDGE keeps them off the HWDGE rings the stream uses
    cdma = nc.gpsimd.dma_start if cc_gp else nc.scalar.dma_start
    # act_light: copies go to DVE so ACT (a HWDGE issuer) stays ~idle
    ccopy = nc.vector.tensor_copy if act_light else nc.scalar.copy

    with ExitStack() as ctx:
        const = ctx.enter_context(tc.tile_pool(name=f"{P_}const", bufs=1))
        idt = const.tile([128, 128], F32)
        cdma(idt[:], ident[:])
        # e8: block j (cols 8j..8j+8) has column j all-ones -> one-hot lhsT
        e8 = const.tile([128, 8 * GRP], F32)
        nc.vector.memset(e8[:], 0.0)
        for j in range(GRP):
            nc.vector.memset(e8[:, 9 * j : 9 * j + 1], 1.0)
        triu_t = const.tile([N_PAIR, N_PAIR], F32)
        cdma(triu_t[:], triu[:])
        cnt_t = const.tile([1, N_PAIR], F32)
        cdma(cnt_t[:], cnt[:])
        negd = const.tile([1, 1], F32)  # -e^{1/tau}: diag correction, post-AR
        nc.vector.memset(negd[:], -float(np.exp(1.0 / TAU)))
        if final_ag or fastfin:
            onesc = const.tile([128, 1], F32)
            nc.vector.memset(onesc[:], 1.0)
        if final_ag:
            negd64 = const.tile([N_PAIR, 1], F32)
            nc.vector.memset(negd64[:], -float(np.exp(1.0 / TAU)))
            cnt_col = const.tile([N_PAIR, 1], F32)
            cdma(cnt_col[:], cnt.rearrange("o b -> b o"))
        if fastfin:
            zeros48 = const.tile([128, 6 * GRP], F32)
            nc.vector.memset(zeros48[:], 0.0)

        # persistent SBUF state (own rows split per group: PE operands must
        # sit at base partition 0)
        zzg = [const.tile([GRP, H], F32, name=f"{P_}zz{g}") for g in range(NG)]
        zLT = const.tile([128, 6 * N_PAIR], F32)  # zhat_L^T chunks
        zzT = const.tile([128, 6 * B_SH], F32)  # zhat_own^T chunks
        sLO = const.tile([N_PAIR, B_SH], F32)  # S_LO columns (SBUF accum)

        dram = ctx.enter_context(tc.tile_pool(name=f"{P_}dram", bufs=1, space="DRAM"))
        shared = "Shared" if use_collective else "Local"
        if warm_cc and use_collective:
            wt = const.tile([1, 1], F32)
            nc.vector.memset(wt[:], 0.0)
            wcc_in = dram.tile([1, 1], F32)
            wcc_out = dram.tile(
                [N_CORES, 1], F32, addr_space="Shared", name=f"{P_}wcc"
            )
            nc.scalar.dma_start(wcc_in[:], wt[:])
            nc.gpsimd.collective_compute(
                "AllGather",
                mybir.AluOpType.bypass,
                replica_groups=[list(range(N_CORES))],
                ins=[wcc_in[:].opt()],
                outs=[wcc_out[:].opt()],
            )
        cc_in = dram.tile([GRP, H], F32)
        ccL = dram.tile([N_PAIR, H], F32, addr_space=shared, name=f"{P_}ccL")
        cc2_in = dram.tile([N_PAIR, 1], F32)
        cc2_shape = [N_CORES * N_PAIR, 1] if final_ag else [N_PAIR, 1]
        cc2_out = dram.tile(cc2_shape, F32, addr_space=shared, name=f"{P_}cc2o")

        xin = ctx.enter_context(tc.tile_pool(name=f"{P_}xin", bufs=xin_bufs))
        psA = ctx.enter_context(tc.tile_pool(name=f"{P_}psA", bufs=2, space="PSUM"))
        # fastfin frees a bank for psZ (group 3 never pools into psA/psB, so
        # bufs=1 only stalls a group boundary by the ~0.1us copy drain)
        psB = ctx.enter_context(
            tc.tile_pool(name=f"{P_}psB", bufs=1 if fastfin else 2, space="PSUM")
        )
        psT = ctx.enter_context(tc.tile_pool(name=f"{P_}psT", bufs=2, space="PSUM"))
        psO = ctx.enter_context(tc.tile_pool(name=f"{P_}psO", bufs=1, space="PSUM"))
        psS = ctx.enter_context(tc.tile_pool(name=f"{P_}psS", bufs=1, space="PSUM"))
        psZ = (
            ctx.enter_context(tc.tile_pool(name=f"{P_}psZ", bufs=1, space="PSUM"))
            if fastfin
            else None
        )
        s2 = ctx.enter_context(tc.tile_pool(name=f"{P_}s2", bufs=1))
        s2t = ctx.enter_context(tc.tile_pool(name=f"{P_}s2t", bufs=2))

        pSLL = psS.tile([N_PAIR, N_PAIR], F32)  # S_LL bank
        # one full psZ bank: cols 0:48 = zzT3 (transposed pooled sums),
        # cols 64:128 = the S_OL matmul output (PSUM tiles are bank-granular)
        zzps = psZ.tile([128, 512], F32, name=f"{P_}zzps") if fastfin else None
        zzT3p = zzps

        def consume_L():
            """Normalize gathered left rows, transpose, S_LL, pair-sum."""
            zL = s2.tile([N_PAIR, H], F32, name=f"{P_}zL")
            # gathered row c*8+j holds logical batch c+8j -> partition 8j+c
            src = ccL.rearrange("(c j) e -> j c e", c=N_CORES)
            cdma(zL[:], src)
            sqs = s2t.tile([N_PAIR, H], F32, tag="sqL")
            nc.vector.tensor_mul(sqs[:], zL[:], zL[:])
            ssn = s2t.tile([N_PAIR, 1], F32, tag="ssL")
            nc.vector.reduce_sum(out=ssn[:], in_=sqs[:], axis=AX.X)
            nrm = s2t.tile([N_PAIR, 1], F32, tag="nrL")
            nc.scalar.activation(nrm[:], ssn[:], AF.Sqrt, scale=TAU)
            rn = s2t.tile([N_PAIR, 1], F32, tag="rnL")
            nc.vector.reciprocal(rn[:], nrm[:])
            nc.vector.tensor_scalar_mul(zL[:], zL[:], rn[:, 0:1])
            for k in range(6):
                pt = psT.tile([128, N_PAIR], F32, tag="pt")
                nc.tensor.transpose(
                    pt[:, 0:N_PAIR],
                    zL[:, k * 128 : (k + 1) * 128],
                    idt[0:N_PAIR, 0:N_PAIR],
                )
                ccopy(zLT[:, k * N_PAIR : (k + 1) * N_PAIR], pt[:, 0:N_PAIR])
            for k in range(6):
                nc.tensor.matmul(
                    pSLL[:],
                    lhsT=zLT[:, k * N_PAIR : (k + 1) * N_PAIR],
                    rhs=zLT[:, k * N_PAIR : (k + 1) * N_PAIR],
                    start=(k == 0),
                    stop=(k == 5),
                )
            mt2 = s2.tile([N_PAIR, N_PAIR], F32, name=f"{P_}mt2")
            nc.vector.tensor_mul(mt2[:], pSLL[:], triu_t[:])
            rs = s2.tile([N_PAIR, 1], F32, name=f"{P_}rs")
            nc.vector.reduce_sum(out=rs[:], in_=mt2[:], axis=AX.X)
            if final_ag:
                return rs  # column finish: no transpose needed
            # transpose to [1,64] so the whole finish chain is single-row
            prT = psT.tile([128, N_PAIR], F32, tag="pt")
            nc.tensor.transpose(prT[0:1, 0:N_PAIR], rs[:], idt[0:N_PAIR, 0:N_PAIR])
            rsT = s2.tile([1, N_PAIR], F32, name=f"{P_}rsT")
            ccopy(rsT[:], prT[0:1, 0:N_PAIR])
            return rsT

        def group_consume(g):
            """Normalize own group rows, transpose, S_LO columns for group g."""
            rows = zzg[g][:]
            sq8 = s2t.tile([GRP, H], F32, tag="sq8")
            nc.vector.tensor_mul(sq8[:], rows, rows)
            sn8 = s2t.tile([GRP, 1], F32, tag="sn8")
            nc.vector.reduce_sum(out=sn8[:], in_=sq8[:], axis=AX.X)
            nr8 = s2t.tile([GRP, 1], F32, tag="nr8")
            nc.scalar.activation(nr8[:], sn8[:], AF.Sqrt, scale=TAU)
            rn8 = s2t.tile([GRP, 1], F32, tag="rn8")
            nc.vector.reciprocal(rn8[:], nr8[:])
            nc.vector.tensor_scalar_mul(rows, rows, rn8[:, 0:1])
            for k in range(6):
                pt = psT.tile([128, N_PAIR], F32, tag="pt")
                nc.tensor.transpose(
                    pt[:, 0:GRP], rows[:, k * 128 : (k + 1) * 128], idt[0:GRP, 0:GRP]
                )
                ccopy(
                    zzT[:, k * B_SH + g * GRP : k * B_SH + (g + 1) * GRP],
                    pt[:, 0:GRP],
                )
            pO = psO.tile([N_PAIR, GRP], F32, tag="pO")
            for k in range(6):
                nc.tensor.matmul(
                    pO[:],
                    lhsT=zLT[:, k * N_PAIR : (k + 1) * N_PAIR],
                    rhs=zzT[:, k * B_SH + g * GRP : k * B_SH + (g + 1) * GRP],
                    start=(k == 0),
                    stop=(k == 5),
                )
            if g < NG - 1:
                ccopy(sLO[:, g * GRP : (g + 1) * GRP], pO[:])
            return pO

        # ---- stage 1: stream + grouped pooling ------------------------------
        if xlayout == "pc":
            x4 = x.rearrange("b (p c) e -> b p c e", c=4)
        else:
            x4 = x.rearrange("b (c p) e -> b p c e", p=128)
        rsT = None
        se_a = [None]
        se_ar = [None]
        pA = pB = None
        for b in range(B_SH):
            g, jg = divmod(b, GRP)
            if "s1" in stages:
                # two half-tile DMAs on the last batch so the chunk-folding
                # adds start when the first half lands (shaves ~2us off the
                # tail); earlier batches use stream_split (bigger DMAs have
                # better HBM efficiency)
                # mix: scalar ring only where ACT compute is guaranteed quiet
                # (consume_L/group_consume land at b=15-17 and 27, exp at 27)
                MIXSB = {1, 3, 5, 7, 9, 11, 13, 21, 23, 25, 29}
                if dma_eng == "alt" and b % 2:
                    eng = nc.scalar
                elif dma_eng == "altg" and b % 2:
                    eng = nc.gpsimd
                elif dma_eng == "mix" and b in MIXSB:
                    eng = nc.scalar
                else:
                    eng = nc.sync
                xt = xin.tile([128, 4 * H], F32)
                if b == B_SH - 1:
                    nsp = 4 if fastfin else 2
                else:
                    nsp = stream_split
                w = 4 * H // nsp
                cw = 4 // nsp
                for s in range(nsp):
                    e_s = eng
                    if dma_eng == "mix" and b == B_SH - 1:
                        # last batch: one half per ring for earliest landing
                        e_s = nc.sync if s == 0 else nc.scalar
                    e_s.dma_start(
                        xt[:, s * w : (s + 1) * w], x4[b, :, s * cw : (s + 1) * cw]
                    )
            if "pool" in stages and fastfin and g == NG - 1:
                # group 3: transposed pooling.  Column 8k+jg of the psZ bank
                # gets pooledT chunk k of this batch via a [128x128]-stationary
                # ones-column matmul, so the tail needs no PSUM->SBUF row copy,
                # no normalize-rows pass, and no PE transposes.
                if jg == 0:
                    # clear the bank once (start=True covers all 48 cols)
                    nc.tensor.matmul(
                        zzT3p[:, 0 : 6 * GRP],
                        lhsT=idt[:],
                        rhs=zeros48[:],
                        start=True,
                        stop=False,
                    )
                if b == B_SH - 1:
                    # quarter-DMAs land progressively: accumulate chunks as
                    # they arrive so only ONE add remains after the last byte
                    nc.vector.tensor_add(xt[:, 0:H], xt[:, 0:H], xt[:, H : 2 * H])
                    nc.vector.tensor_add(
                        xt[:, 0:H], xt[:, 0:H], xt[:, 2 * H : 3 * H]
                    )
                    nc.vector.tensor_add(
                        xt[:, 0:H], xt[:, 0:H], xt[:, 3 * H : 4 * H]
                    )
                else:
                    nc.vector.tensor_add(xt[:, 0:H], xt[:, 0:H], xt[:, H : 2 * H])
                    nc.vector.tensor_add(
                        xt[:, 2 * H : 3 * H],
                        xt[:, 2 * H : 3 * H],
                        xt[:, 3 * H : 4 * H],
                    )
                    nc.vector.tensor_add(
                        xt[:, 0:H], xt[:, 0:H], xt[:, 2 * H : 3 * H]
                    )
                for k in range(6):
                    nc.tensor.matmul(
                        zzT3p[:, 8 * k + jg : 8 * k + jg + 1],
                        lhsT=xt[:, k * 128 : (k + 1) * 128],
                        rhs=onesc[:, 0:1],
                        start=False,
                        stop=True,
                    )
            elif "pool" in stages:
                if jg == 0:
                    pA = psA.tile([GRP, 512], F32, tag="A")
                    pB = psB.tile([GRP, 256], F32, tag="B")
                lw = e8[:, GRP * jg : GRP * (jg + 1)]
                if b == B_SH - 1:
                    # last batch: fold only half 1 on DVE; chunks 2 and 3 go
                    # straight into the (warm) PE accumulation.  Shorter tail
                    # path than add2 -> add3 -> matmul, and drops the DVE->PE
                    # handoff from the critical path.  Emission order: chunks
                    # 2,3 first (only need the half-2 DMA), folded chunk last.
                    nc.vector.tensor_add(xt[:, 0:H], xt[:, 0:H], xt[:, H : 2 * H])
                    for base in (2 * H, 3 * H, 0):
                        nc.tensor.matmul(
                            pA[:, :],
                            lhsT=lw,
                            rhs=xt[:, base : base + 512],
                            start=False,
                            stop=(base == 0),
                        )
                    for base in (2 * H, 3 * H, 0):
                        nc.tensor.matmul(
                            pB[:, :],
                            lhsT=lw,
                            rhs=xt[:, base + 512 : base + H],
                            start=False,
                            stop=(base == 0),
                        )
                else:
                    # fold the 4 sequence chunks on the (otherwise idle) DVE
                    # so PE streams 768 cols/batch instead of 3072 (PE at
                    # cold 1.2GHz was the stream bottleneck at 8 mm/batch)
                    nc.vector.tensor_add(xt[:, 0:H], xt[:, 0:H], xt[:, H : 2 * H])
                    nc.vector.tensor_add(
                        xt[:, 2 * H : 3 * H],
                        xt[:, 2 * H : 3 * H],
                        xt[:, 3 * H : 4 * H],
                    )
                    nc.vector.tensor_add(xt[:, 0:H], xt[:, 0:H], xt[:, 2 * H : 3 * H])
                    nc.tensor.matmul(
                        pA[:, :],
                        lhsT=lw,
                        rhs=xt[:, 0:512],
                        start=(jg == 0),
                        stop=(jg == GRP - 1),
                    )
                    nc.tensor.matmul(
                        pB[:, :],
                        lhsT=lw,
                        rhs=xt[:, 512:H],
                        start=(jg == 0),
                        stop=(jg == GRP - 1),
                    )
                if jg == GRP - 1:
                    # split across ACT and DVE so the two bank copies run in
                    # parallel (group 3's copies sit on the exposed tail);
                    # act_light puts both on DVE to keep the ACT ring clean
                    if act_light:
                        nc.vector.tensor_copy(zzg[g][:, 0:512], pA[:])
                    else:
                        nc.scalar.copy(zzg[g][:, 0:512], pA[:])
                    nc.vector.tensor_copy(zzg[g][:, 512:H], pB[:])
            if "cc" in stages:
                if b == GRP - 1:
                    cdma(cc_in[:], zzg[0][:])
                    if use_collective:
                        nc.gpsimd.collective_compute(
                            "AllGather",
                            mybir.AluOpType.bypass,
                            replica_groups=[list(range(N_CORES))],
                            ins=[cc_in[:].opt()],
                            outs=[ccL[:].opt()],
                        )
                    else:
                        for c in range(N_CORES):
                            nc.scalar.dma_start(
                                ccL[c * GRP : (c + 1) * GRP, :], cc_in[:]
                            )
                if "s2" in stages:
                    if b == 15:
                        rsT = consume_L()
                    elif b == 16:
                        group_consume(0)
                    elif b == 17:
                        group_consume(1)
                    elif b == 27:
                        group_consume(2)
                        # exp for groups 0..2 hidden under the stream
                        eta = s2.tile([N_PAIR, 3 * GRP], F32, name=f"{P_}eta")
                        sa = s2.tile([N_PAIR, 1], F32, name=f"{P_}sea")
                        nc.scalar.activation(
                            eta[:],
                            sLO[:, 0 : 3 * GRP],
                            AF.Exp,
                            scale=1.0,
                            accum_out=sa[:],
                        )
                        se_a[0] = sa
                        if fastfin:
                            # row form for the 1-descriptor collective-input
                            # DMA at the tail (hidden here under the stream)
                            saT = psT.tile([128, N_PAIR], F32, tag="pt")
                            nc.tensor.transpose(
                                saT[0:1, 0:N_PAIR],
                                sa[:],
                                idt[0:N_PAIR, 0:N_PAIR],
                            )
                            sa_row = s2.tile([1, N_PAIR], F32, name=f"{P_}sarow")
                            ccopy(sa_row[:], saT[0:1, 0:N_PAIR])
                            se_ar[0] = sa_row
            if "pool" in stages and "s2" in stages and warm and b == B_SH - 2:
                # ~3.5us of back-to-back dummy PE work, hidden under batch
                # 31's DMA window: trips the HAM activity monitor (4096-cycle
                # window) so the tail's matmuls/transposes run at the warm
                # 2.4GHz clock instead of the cold 1.2GHz default
                for _ in range(30):
                    ptw = psT.tile([128, N_PAIR], F32, tag="pt")
                    nc.tensor.transpose(
                        ptw[:, 0:N_PAIR],
                        idt[0:N_PAIR, :],
                        idt[0:N_PAIR, 0:N_PAIR],
                    )

        if "cc" not in stages or "s2" not in stages:
            return
        if skip_finish:
            fin_cut = "pre"
        if fastfin:
            if fin_cut == "pre":
                return
            # ---- fastfin tail: zzT3 (PSUM, transposed raw sums) -> S_OL ----
            # one [128,48] copy replaces the row copy + normalize-rows +
            # 6 transposes of group_consume; norms via ones-column matmuls.
            zzT3sb = s2.tile([128, 6 * GRP], F32, name=f"{P_}zzT3sb")
            nc.scalar.copy(zzT3sb[:], zzT3p[:, 0 : 6 * GRP])
            # S_OL[j, i] = zraw_own_j . zhat_L_i (normalize by rn8f after);
            # emitted FIRST so PE's long pole starts as soon as the copy
            # lands.  Output shares the psZ bank; its start=True clear of
            # zzT3 is safe (zzT3's only reader, the zzT3sb copy, is upstream
            # of these matmuls).
            pOL = zzps[0:GRP, 64 : 64 + N_PAIR]
            for k in range(6):
                nc.tensor.matmul(
                    pOL,
                    lhsT=zzT3sb[:, 8 * k : 8 * (k + 1)],
                    rhs=zLT[:, k * N_PAIR : (k + 1) * N_PAIR],
                    start=(k == 0),
                    stop=(k == 5),
                )
            sq48 = s2t.tile([128, 6 * GRP], F32, tag="sq48")
            nc.vector.tensor_mul(sq48[:], zzT3sb[:], zzT3sb[:])
            pn = psT.tile([128, N_PAIR], F32, tag="pt")
            for k in range(6):
                nc.tensor.matmul(
                    pn[0:GRP, 0:1],
                    lhsT=sq48[:, 8 * k : 8 * (k + 1)],
                    rhs=onesc[:, 0:1],
                    start=(k == 0),
                    stop=(k == 5),
                )
            nr8f = s2t.tile([GRP, 1], F32, tag="nr8f")
            nc.scalar.activation(nr8f[:], pn[0:GRP, 0:1], AF.Sqrt, scale=TAU)
            rn8f = s2t.tile([GRP, 1], F32, tag="rn8f")
            nc.vector.reciprocal(rn8f[:], nr8f[:])
            sOL = s2.tile([GRP, N_PAIR], F32, name=f"{P_}sOL")
            nc.vector.tensor_scalar_mul(sOL[:], pOL, rn8f[:, 0:1])
            eOL = s2.tile([GRP, N_PAIR], F32, name=f"{P_}eOL")
            nc.scalar.activation(eOL[:], sOL[:], AF.Exp, scale=1.0)
            # D3 straight to row form: with the (hidden) se_a row, the
            # collective-input DMA is a single 256B descriptor
            pd3r = psT.tile([128, N_PAIR], F32, tag="pt")
            nc.tensor.matmul(
                pd3r[0:1, 0:N_PAIR],
                lhsT=onesc[0:GRP, 0:1],
                rhs=eOL[:],
                start=True,
                stop=True,
            )
            se_row = s2.tile([1, N_PAIR], F32, name=f"{P_}serow")
            nc.vector.tensor_add(se_row[:], se_ar[0], pd3r[0:1, 0:N_PAIR])
        else:
            pO3 = group_consume(3)
            if fin_cut == "pre":
                return

            # ---- finish: partial denominators -> AllReduce -> loss ----------
            # exp over group-3 columns straight from PSUM; groups 0..2 were
            # exp'd mid-stream (exp_a).  se = se_a + se_b, diag removed later.
            et = s2.tile([N_PAIR, GRP], F32, name=f"{P_}et")
            se_b = s2.tile([N_PAIR, 1], F32, name=f"{P_}seb")
            nc.scalar.activation(et[:], pO3[:], AF.Exp, scale=1.0, accum_out=se_b[:])
            se = s2.tile([N_PAIR, 1], F32, name=f"{P_}se")
            nc.vector.tensor_add(se[:], se_a[0], se_b[:])
        if fin_cut == "se":
            return
        if fastfin:
            nc.scalar.dma_start(cc2_in[:].rearrange("a b -> b a"), se_row[:])
        else:
            nc.scalar.dma_start(cc2_in[:], se[:])
        if fin_cut == "dma":
            return
        if use_collective:
            nc.gpsimd.collective_compute(
                "AllGather" if final_ag else "AllReduce",
                mybir.AluOpType.bypass if final_ag else mybir.AluOpType.add,
                replica_groups=[list(range(N_CORES))],
                ins=[cc2_in[:].opt()],
                outs=[cc2_out[:].opt()],
            )
        elif final_ag:
            for c in range(N_CORES):
                nc.scalar.dma_start(
                    cc2_out[c * N_PAIR : (c + 1) * N_PAIR, :], cc2_in[:]
                )
        else:
            nc.scalar.dma_start(cc2_out[:], cc2_in[:])
        if fin_cut == "cc":
            return
        if final_ag:
            # column finish: partition c <- core c's [64] (8 contiguous
            # descriptors), PE ones-matmul sums the cores, then the whole
            # chain stays [64,1] (no transposes; rsT here is the rs column).
            l8 = s2.tile([N_CORES, N_PAIR], F32, name=f"{P_}l8")
            nc.scalar.dma_start(
                l8[:], cc2_out[:].rearrange("(c i) o -> c (i o)", c=N_CORES)
            )
            pD = psT.tile([128, N_PAIR], F32, tag="pt")
            nc.tensor.matmul(
                pD[0:N_PAIR, 0:1],
                lhsT=l8[:],
                rhs=onesc[0:N_CORES, 0:1],
                start=True,
                stop=True,
            )
            ldc = s2.tile([N_PAIR, 1], F32, name=f"{P_}ldc")
            nc.scalar.activation(ldc[:], pD[0:N_PAIR, 0:1], AF.Ln, bias=negd64[:])
            t1c = s2.tile([N_PAIR, 1], F32, name=f"{P_}t1c")
            nc.vector.tensor_mul(t1c[:], ldc[:], cnt_col[:])
            prc = s2.tile([N_PAIR, 1], F32, name=f"{P_}prc")
            nc.vector.tensor_sub(prc[:], t1c[:], rsT[:])
            pres = psT.tile([128, N_PAIR], F32, tag="pt")
            nc.tensor.matmul(
                pres[0:1, 0:1],
                lhsT=prc[:],
                rhs=onesc[0:N_PAIR, 0:1],
                start=True,
                stop=True,
            )
            res = s2.tile([1, 1], F32, name=f"{P_}res")
            nc.scalar.copy(res[:], pres[0:1, 0:1])
            nc.sync.dma_start(out[0:1, 0:1], res[:])
            return
        # single-row finish: read D back as [1,64], remove the diagonal
        # (each row owned by exactly one core -> sum of e^{1/tau} once)
        # as a constant Ln bias, then weighted-accumulate to the scalar.
        lds = s2.tile([1, N_PAIR], F32, name=f"{P_}lds")
        nc.scalar.dma_start(lds[:], cc2_out[:].rearrange("a b -> b a"))
        ld = s2.tile([1, N_PAIR], F32, name=f"{P_}ld")
        nc.scalar.activation(ld[:], lds[:], AF.Ln, bias=negd[:])
        t1 = s2.tile([1, N_PAIR], F32, name=f"{P_}t1")
        nc.vector.tensor_mul(t1[:], ld[:], cnt_t[:])
        pr = s2.tile([1, N_PAIR], F32, name=f"{P_}pr")
        nc.vector.tensor_sub(pr[:], t1[:], rsT[:])
        junk = s2.tile([1, N_PAIR], F32, name=f"{P_}junk")
        res = s2.tile([1, 1], F32, name=f"{P_}res")
        nc.scalar.activation(junk[:], pr[:], AF.Copy, accum_out=res[:])
        nc.scalar.dma_start(out[0:1, 0:1], res[:])


KERNEL_VERSION = "v2"


def build_nc(reps=1, version=None, serialize_reps=False, **body_kwargs):
    version = version or KERNEL_VERSION
    nc = bacc.Bacc("TRN2", target_bir_lowering=False, debug=False, num_devices=N_CORES)
    x = nc.dram_tensor("x", [B_SH, S, H], F32, kind="ExternalInput")
    ident = nc.dram_tensor("ident", [128, 128], F32, kind="ExternalInput")
    triu = nc.dram_tensor("triu", [N_PAIR, N_PAIR], F32, kind="ExternalInput")
    if version == "v1":
        cnt = nc.dram_tensor("cnt", [N_PAIR, 1], F32, kind="ExternalInput")
        dmask = nc.dram_tensor("dmask", [N_PAIR, B], F32, kind="ExternalInput")
    else:
        cnt = nc.dram_tensor("cnt", [1, N_PAIR], F32, kind="ExternalInput")
    out = nc.dram_tensor("loss", [1, 1], F32, kind="ExternalOutput")
    with tile.TileContext(nc) as tc:
        for r in range(reps):
            prefix = f"r{r}_" if reps > 1 else ""
            if serialize_reps and r > 0:
                # block this rep's stream-issue rings on the previous rep's
                # final out-write so K-diff measures honest serial per-rep
                # time (no cross-rep overlap games)
                with tc.tile_pool(name=f"ser{r}", bufs=1) as serp:
                    tok = serp.tile([1, 2], F32, name=f"tok{r}")
                    nc.sync.dma_start(tok[0:1, 0:1], out.ap()[0:1, 0:1])
                    nc.scalar.dma_start(tok[0:1, 1:2], out.ap()[0:1, 0:1])
            if version == "v1":
                _body(
                    tc,
                    x.ap(),
                    ident.ap(),
                    dmask.ap(),
                    triu.ap(),
                    cnt.ap(),
                    out.ap(),
                    prefix=prefix,
                    **body_kwargs,
                )
            else:
                _body_v2(
                    tc,
                    x.ap(),
                    ident.ap(),
                    triu.ap(),
                    cnt.ap(),
                    out.ap(),
                    prefix=prefix,
                    **body_kwargs,
                )
    nc.compile()
    return nc


def const_inputs(version=None):
    version = version or KERNEL_VERSION
    ident = np.eye(128, dtype=np.float32)
    triu = np.triu(np.ones((N_PAIR, N_PAIR), dtype=np.float32), k=1)
    cnt = (N_PAIR - 1 - np.arange(N_PAIR, dtype=np.float32)).reshape(N_PAIR, 1)
    if version == "v1":
        dmask = np.zeros((N_PAIR, B), dtype=np.float32)
        dmask[np.arange(N_PAIR), np.arange(N_PAIR)] = NEG
        return {"ident": ident, "triu": triu, "cnt": cnt, "dmask": dmask}
    # v2: fold the final -2/n*(n-1) scale into triu and cnt so the loss is
    # a plain accumulate after the AllReduce
    sc = -2.0 / N_PAIR * (N_PAIR - 1)
    return {
        "ident": ident,
        "triu": (sc * triu).astype(np.float32),
        "cnt": (sc * cnt).astype(np.float32).reshape(1, N_PAIR),
    }


def make_in_maps(last_hidden_states, input_mask, version=None):
    version = version or KERNEL_VERSION
    del input_mask  # cancels exactly in the L2 normalization (see half_tail)
    x = np.asarray(last_hidden_states, dtype=np.float32)
    consts = const_inputs(version)
    return [
        {"x": np.ascontiguousarray(x[c::N_CORES]), **consts}  # logical c+8j
        for c in range(N_CORES)
    ]


_CACHE = {}


def get_nc(version=None):
    key = version or KERNEL_VERSION
    if key not in _CACHE:
        _CACHE[key] = build_nc(version=key)
    return _CACHE[key]


def kernel(last_hidden_states, input_mask):
    nc = get_nc()
    in_maps = make_in_maps(last_hidden_states, input_mask)
    res = bass_utils.run_bass_kernel_spmd(nc, in_maps, core_ids=list(range(N_CORES)))
    return np.asarray(res.results[0]["loss"], dtype=np.float32).reshape(())



# revision 29
# speedup vs baseline: 1.2019x; 1.0423x over previous
"""Trainium2 Bass kernel: BertCL mean-pool + NT-Xent contrastive loss.

Contract: kernel(last_hidden_states [256,512,768] f32, input_mask [256,512] f32)
-> scalar f32 loss, numerically matching the jax reference.

Strategy (8 NeuronCores, SPMD), "v2" (see _body_v2; v1 kept for reference):
  Batch axis sharded STRIDED: core c owns logical batches {c, c+8, ...}
  (local j <-> logical c + 8j).  Only rows 0..63 ("left" rows) of z enter
  the loss as logsumexp rows / pair terms, and under the strided sharding
  those are exactly locals j<8 of every core.

  stage 1 (memory-bound, ~140.6us/core HBM roofline): per batch, stream
    [512,768] as a [128, 4*768] tile ("(p c) e" layout -> 12KB contiguous
    runs per partition, two half-tile DMAs), fold the 4 sequence chunks on
    the otherwise-idle DVE (PE at cold 1.2GHz was the bottleneck when it
    streamed all 3072 columns), then one-hot-lhsT matmuls accumulate
    8-batch groups into [8,512]+[8,256] PSUM banks -> pooled rows land
    partition-aligned in SBUF.  Mask division cancels in L2-normalization
    and is skipped.
  Hidden mid-stream: after group 0 (locals 0..7 = logical 0..63 across
    cores) an AllGather ships raw sums; each core normalizes the gathered
    [64,768] (1/tau folded into the norm), PE-transposes it, computes
    S_LL = zL zL^T, the pair-sum term, and per own-group S_LO columns
    zL . zhat_own -- all overlapped with the remaining DMA stream.
  Tail (the only exposed part), "fastfin": group 3 is pooled TRANSPOSED
    (per batch, 6 ones-column matmuls write pooledT chunk columns into a
    PSUM bank), so the tail needs one [128,48] PSUM->SBUF copy, norms via
    ones-matmuls, S_OL = zzT3^T zLT directly (no row-normalize pass, no
    PE transposes), exp + ones-matmul -> per-core partial denominators
    D_i^c = sum_{k in own rows} exp(S_ik), kept in ROW form ([1,64]; the
    se_a transpose hides at b=27) so the collective-input DMA is a single
    256B descriptor, and the last batch streams as 4 quarter-DMAs with
    incremental folds (one DVE add left after the last byte).  A
    [64]-float AllGather (cheaper than AllReduce) ships partials; the
    post chain stays in [64,1] column form: PE ones-matmul sums the 8
    cores, Ln with the diagonal removed as a constant per-partition bias
    (each row is owned exactly once), weighted accumulate (triu/cnt
    pre-scaled by -2/n*(n-1)) -> scalar.
  warm_cc=True adds a dummy 4-byte AllGather at t=0 that absorbs the
    cold-start collective setup under the stream on a cold single-shot
    run; it costs ~3us per iteration steady-state, so it is off by
    default.

  Tuning (serialized K-differential, see perf_lab.py/lab.py, 2026-08-10):
  ~176-182us vs ~192us for the previous config and the 140.6us HBM
  roofline.  Key measured facts: the stream must issue from the SINGLE
  sync HWDGE ring - any dual-ring scheme (sync/scalar alternation, mix,
  SWDGE) costs 10-25us in the full kernel despite helping a DMA-only
  stream; xin_bufs=12 (vs 6) saves ~10us of issue stalls; fastfin +
  AllGather-finish cuts the exposed tail from ~33us to ~20us.  Relative
  error vs fp32 jax: 2.7e-7 on HW.

  NOTE: fused DVE ops (tensor_tensor_reduce, scalar_tensor_tensor) pass
  CoreSim but hang/crash this hardware - only plain DVE ops are used.
  PE operand APs must start at partition 0/32/64; matmul accumulation
  start=True clears the whole PSUM bank (PSUM tiles are bank-granular:
  zzT3/S_OL share one bank with transitive cross-engine ordering making
  the bank clears safe).
"""

import sys
from contextlib import ExitStack

import numpy as np

_REPO = "/opt/trn_rl_repo"
if _REPO not in sys.path:
    sys.path.insert(0, _REPO)

import concourse.bass as bass  # noqa: E402  (kept for callers/debugging)
import concourse.tile as tile  # noqa: E402
from concourse import bacc, bass_utils, mybir  # noqa: E402

N_CORES = 8
B, S, H = 256, 512, 768
B_SH = B // N_CORES  # 32 local batches per core
HALF = B_SH // 2  # 16
N_PAIR = B // 4  # 64
TAU = 0.5
F32 = mybir.dt.float32
AX = mybir.AxisListType
AF = mybir.ActivationFunctionType
NEG = -30000.0  # diagonal mask value; exp(NEG + logit) == 0 exactly in fp32


def _body(
    tc,
    x,
    ident,
    dmask,
    triu,
    cnt,
    out,
    use_collective=True,
    stages=("s1", "pool", "cc", "s2"),
    prefix="",
    xlayout="cp",
):
    nc = tc.nc
    P_ = prefix

    with ExitStack() as ctx:
        const = ctx.enter_context(tc.tile_pool(name=f"{P_}const", bufs=1))
        ones_col = const.tile([128, 1], F32)
        nc.vector.memset(ones_col[:], 1.0)
        idt = const.tile([128, 128], F32)
        nc.sync.dma_start(idt[:], ident[:])

        dram = ctx.enter_context(tc.tile_pool(name=f"{P_}dram", bufs=1, space="DRAM"))
        cc_in = dram.tile([B_SH, H], F32)
        shared = "Shared" if use_collective else "Local"
        # asymmetric split: gather locals [0,24) early (hides under the last 8
        # batches' streaming), locals [24,32) at the end (only 64 logical rows
        # of consume work left after the final latency-bound collective)
        SEG = [(0, 16), (16, 32)]
        cc_o = [
            dram.tile([8 * (j1 - j0), H], F32, addr_space=shared, name=f"{P_}cc_o{h}")
            for h, (j0, j1) in enumerate(SEG)
        ]

        # staging row for pooled sums: [1, 32*768] on partition 0
        pooled_sb = const.tile([1, B_SH * H], F32)

        xin = ctx.enter_context(tc.tile_pool(name=f"{P_}xin", bufs=6))
        ps1 = ctx.enter_context(tc.tile_pool(name=f"{P_}ps1", bufs=2, space="PSUM"))
        s2 = ctx.enter_context(tc.tile_pool(name=f"{P_}s2", bufs=1))
        s2t = ctx.enter_context(tc.tile_pool(name=f"{P_}s2t", bufs=2))
        psT = ctx.enter_context(tc.tile_pool(name=f"{P_}psT", bufs=2, space="PSUM"))
        psS = ctx.enter_context(tc.tile_pool(name=f"{P_}psS", bufs=1, space="PSUM"))

        # zT[:, k*256 + p] = z[p, k*128 + q] for partition q (h on partitions)
        zT = s2.tile([128, 6 * B], F32)
        pS = psS.tile([N_PAIR, B], F32)

        def send_half(h):
            """Gather raw sums for local rows [16h,16h+16).

            The reference divides pooled sums by the mask row-sum before
            L2-normalizing; that per-row positive scalar cancels exactly in
            the normalization, so we gather raw sums and normalize the
            gathered rows (same result to ~1ulp, and the pre-collective
            tail shrinks to a single DMA)."""
            j0, j1 = SEG[h]
            nc.sync.dma_start(
                cc_in[j0:j1, :],
                pooled_sb[0:1, j0 * H : j1 * H].rearrange("o (b e) -> o b e", e=H),
            )

            if use_collective:
                nc.gpsimd.collective_compute(
                    "AllGather",
                    mybir.AluOpType.bypass,
                    replica_groups=[list(range(N_CORES))],
                    ins=[cc_in[j0:j1, :].opt()],
                    outs=[cc_o[h].opt()],
                )
            else:
                n = j1 - j0
                for c in range(N_CORES):
                    nc.sync.dma_start(
                        cc_o[h][c * n : (c + 1) * n, :], cc_in[j0:j1, :]
                    )

        def consume_block(h, ja, jb, name):
            """Normalize logical rows [8*ja, 8*jb) from gather h; fill zT cols.

            Gathered row (c, j - SEG[h][0]) holds logical batch c + 8j; the
            permuted 3-D AP (j, c, e) lands partitions in logical order."""
            P = 8 * (jb - ja)  # rows in this block
            col = 8 * ja  # zT column base = first logical row
            zh = s2.tile([P, H], F32, tag=name, name=name)
            src = cc_o[h].rearrange("(c j) e -> j c e", c=N_CORES)
            nc.sync.dma_start(zh[:], src[ja - SEG[h][0] : jb - SEG[h][0]])
            sqs = s2t.tile([P, H], F32, tag=f"sqs{name}", name=f"sqs{name}")
            ssn = s2t.tile([P, 1], F32, tag=f"ssn{name}", name=f"ssn{name}")
            nc.vector.tensor_mul(sqs[:], zh[:], zh[:])
            nc.vector.reduce_sum(out=ssn[:], in_=sqs[:], axis=AX.X)
            # sqrt(TAU * ss): scales z by 1/sqrt(tau) so S = z'z'^T = logits
            nrm = s2t.tile([P, 1], F32, tag=f"nrm{name}", name=f"nrm{name}")
            nc.scalar.activation(nrm[:], ssn[:], AF.Sqrt, scale=TAU)
            rn = s2t.tile([P, 1], F32, tag=f"rn{name}", name=f"rn{name}")
            nc.vector.reciprocal(rn[:], nrm[:])
            nc.vector.tensor_scalar_mul(zh[:], zh[:], rn[:, 0:1])
            for k in range(6):
                pt = psT.tile([128, 128], F32, tag="pt")
                nc.tensor.transpose(
                    pt[:, 0:P], zh[:, k * 128 : (k + 1) * 128], idt[0:P, 0:P]
                )
                nc.vector.tensor_copy(
                    zT[:, k * B + col : k * B + col + P], pt[:, 0:P]
                )

        def logits_block(col, n):
            """S[0:64, col:col+n] += sum_k zT_k[:, 0:64].T @ zT_k[:, col:col+n]"""
            for k in range(6):
                nc.tensor.matmul(
                    pS[:, col : col + n],
                    lhsT=zT[:, k * B : k * B + N_PAIR],
                    rhs=zT[:, k * B + col : k * B + col + n],
                    start=(k == 0),
                    stop=(k == 5),
                )

        # ---- stage 1: per-batch sum over the sequence axis -------------------
        if xlayout == "pc":
            # partition p <- rows 4p..4p+3: contiguous 12KB DMA runs/partition
            x4 = x.rearrange("b (p c) e -> b p c e", c=4)  # [32, 128, 4, 768]
        else:
            # partition p <- rows p, 128+p, ...: 4x 3KB runs/partition
            x4 = x.rearrange("b (c p) e -> b p c e", p=128)  # [32, 128, 4, 768]
        for b in range(B_SH):
            if "s1" in stages:
                xt = xin.tile([128, 4 * H], F32)
                nc.sync.dma_start(xt[:], x4[b])
            if "pool" in stages:
                ps = ps1.tile([1, H], F32)
                for c in range(4):
                    nc.tensor.matmul(
                        ps[:, 0:512],
                        lhsT=ones_col[:, 0:1],
                        rhs=xt[:, c * H : c * H + 512],
                        start=(c == 0),
                        stop=(c == 3),
                    )
                for c in range(4):
                    nc.tensor.matmul(
                        ps[:, 512:H],
                        lhsT=ones_col[:, 0:1],
                        rhs=xt[:, c * H + 512 : (c + 1) * H],
                        start=(c == 0),
                        stop=(c == 3),
                    )
                nc.scalar.copy(pooled_sb[0:1, b * H : (b + 1) * H], ps[:])
            if "cc" in stages:
                if b == SEG[0][1] - 1:
                    send_half(0)
                elif b == SEG[1][1] - 1:
                    send_half(1)

        if "cc" not in stages or "s2" not in stages:
            return
        # each gather carries a full 128-row half of z
        consume_block(0, 0, 16, "zb0")
        logits_block(0, 128)
        consume_block(1, 16, 32, "zb1")
        logits_block(128, 128)

        # ---- finish: masked logsumexp + pair sum ----------------------------
        # pS already holds logits (1/tau folded into the normalization)
        dm = s2.tile([N_PAIR, B], F32)
        nc.sync.dma_start(dm[:], dmask[:])
        sd = s2.tile([N_PAIR, B], F32)
        nc.vector.tensor_add(sd[:], pS[:], dm[:])

        # logits are cosine/tau in [-2,2]: exp is safe without max-subtraction
        et = s2.tile([N_PAIR, B], F32)
        se = s2.tile([N_PAIR, 1], F32)
        nc.scalar.activation(et[:], sd[:], AF.Exp, scale=1.0, accum_out=se[:])
        ld = s2.tile([N_PAIR, 1], F32)
        nc.scalar.activation(ld[:], se[:], AF.Ln)  # logden

        # sum_{i<j<n} (logden[i] - logits[i,j])
        #   = sum_i cnt[i]*logden[i] - sum_ij triu[i,j]*logits[i,j]
        tri_t = s2.tile([N_PAIR, N_PAIR], F32)
        nc.sync.dma_start(tri_t[:], triu[:])
        cnt_t = s2.tile([N_PAIR, 1], F32)
        nc.sync.dma_start(cnt_t[:], cnt[:])
        mt2 = s2.tile([N_PAIR, N_PAIR], F32)
        nc.vector.tensor_mul(mt2[:], sd[0:N_PAIR, 0:N_PAIR], tri_t[:])
        rs = s2.tile([N_PAIR, 1], F32)
        nc.vector.reduce_sum(out=rs[:], in_=mt2[:], axis=AX.X)
        t1 = s2.tile([N_PAIR, 1], F32)
        nc.vector.tensor_mul(t1[:], ld[:], cnt_t[:])
        pr = s2.tile([N_PAIR, 1], F32)
        nc.vector.tensor_sub(pr[:], t1[:], rs[:])

        ptot = psS.tile([1, 1], F32, tag="ptot")
        nc.tensor.matmul(
            ptot[:], lhsT=pr[:], rhs=ones_col[0:N_PAIR, 0:1], start=True, stop=True
        )
        res = s2.tile([1, 1], F32)
        nc.vector.tensor_scalar_mul(res[:], ptot[:], -2.0 / N_PAIR * (N_PAIR - 1))
        nc.sync.dma_start(out[0:1, 0:1], res[:])


def _body_v2(
    tc,
    x,
    ident,
    triu,
    cnt,
    out,
    use_collective=True,
    stages=("s1", "pool", "cc", "s2"),
    prefix="",
    xlayout="pc",
    final_ag=True,  # AllGather + local reduce beats AllReduce (shorter finish)
    stream_split=2,  # DMAs per batch for b < 31 (batch 31 keeps its 2-half path)
    dma_eng="sync",  # "sync" | "alt" (sync/scalar) | "altg" (sync/gpsimd) | "mix"
    cc_gp=False,  # consts + consume DMAs via SWDGE (keeps HWDGE rings clean)
    skip_finish=False,  # drop the final collective + loss chain (timing decomp)
    fin_cut=None,  # None|"pre"|"se"|"dma"|"cc": truncate the finish (decomp)
    warm=True,  # PE warm-up spam before the tail
    fastfin=True,  # group 3 via transposed pooling -> S_OL (shorter tail chain)
    act_light=False,  # move ACT copies to DVE (frees the scalar HWDGE ring)
    xin_bufs=12,  # stream double-buffer depth (12 absorbs consume-phase stalls)
    warm_cc=False,  # tiny AllGather at t=0: absorbs cold-collective setup on a
    # cold single-shot run, but costs ~3us/rep steady-state - off by default
):
    """v2: the only exposed collective is a [64]-float AllReduce.

    Strided sharding (core c owns logical batches c+8j) makes logical rows
    0..63 exactly the locals j<8 of every core, so the "left" rows the loss
    needs are gathered ~25% into the stream (fully hidden).  Each core then
    forms partial NT-Xent denominators D_i^c = sum_{k in own 32 rows}
    exp(zhat_i . zhat_k) locally (its own rows never need to be gathered),
    and the final collective is AllReduce(D) of 64 floats + a ~2us chain.
    The diagonal term exp(zhat_i.zhat_i)=e^2 appears exactly once across
    cores, so it is removed AFTER the AllReduce as a constant Ln bias.
    triu and cnt arrive pre-scaled by -2/n*(n-1) so the loss is a plain
    accumulate at the end (no final matmul / rescale).

    Pooling accumulates 8-batch groups into [8,512]+[8,256] PSUM banks via
    one-hot lhsT columns, so pooled rows land partition-aligned (no
    single-row staging, no DRAM round trip for the own-row path).
    """
    nc = tc.nc
    P_ = prefix
    GRP = 8  # batches per pooling group
    NG = B_SH // GRP  # 4 groups
    # small/consume DMAs: SWDGE keeps them off the HWDGE rings the stream uses
    cdma = nc.gpsimd.dma_start if cc_gp else nc.scalar.dma_start
    # act_light: copies go to DVE so ACT (a HWDGE issuer) stays ~idle
    ccopy = nc.vector.tensor_copy if act_light else nc.scalar.copy

    with ExitStack() as ctx:
        const = ctx.enter_context(tc.tile_pool(name=f"{P_}const", bufs=1))
        idt = const.tile([128, 128], F32)
        cdma(idt[:], ident[:])
        # e8: block j (cols 8j..8j+8) has column j all-ones -> one-hot lhsT
        e8 = const.tile([128, 8 * GRP], F32)
        nc.vector.memset(e8[:], 0.0)
        for j in range(GRP):
            nc.vector.memset(e8[:, 9 * j : 9 * j + 1], 1.0)
        triu_t = const.tile([N_PAIR, N_PAIR], F32)
        cdma(triu_t[:], triu[:])
        cnt_t = const.tile([1, N_PAIR], F32)
        cdma(cnt_t[:], cnt[:])
        negd = const.tile([1, 1], F32)  # -e^{1/tau}: diag correction, post-AR
        nc.vector.memset(negd[:], -float(np.exp(1.0 / TAU)))
        if final_ag or fastfin:
            onesc = const.tile([128, 1], F32)
            nc.vector.memset(onesc[:], 1.0)
        if final_ag:
            negd64 = const.tile([N_PAIR, 1], F32)
            nc.vector.memset(negd64[:], -float(np.exp(1.0 / TAU)))
            cnt_col = const.tile([N_PAIR, 1], F32)
            cdma(cnt_col[:], cnt.rearrange("o b -> b o"))
        if fastfin:
            zeros48 = const.tile([128, 6 * GRP], F32)
            nc.vector.memset(zeros48[:], 0.0)

        # persistent SBUF state (own rows split per group: PE operands must
        # sit at base partition 0)
        zzg = [const.tile([GRP, H], F32, name=f"{P_}zz{g}") for g in range(NG)]
        zLT = const.tile([128, 6 * N_PAIR], F32)  # zhat_L^T chunks
        zzT = const.tile([128, 6 * B_SH], F32)  # zhat_own^T chunks
        sLO = const.tile([N_PAIR, B_SH], F32)  # S_LO columns (SBUF accum)

        dram = ctx.enter_context(tc.tile_pool(name=f"{P_}dram", bufs=1, space="DRAM"))
        shared = "Shared" if use_collective else "Local"
        if warm_cc and use_collective:
            wt = const.tile([1, 1], F32)
            nc.vector.memset(wt[:], 0.0)
            wcc_in = dram.tile([1, 1], F32)
            wcc_out = dram.tile(
                [N_CORES, 1], F32, addr_space="Shared", name=f"{P_}wcc"
            )
            nc.scalar.dma_start(wcc_in[:], wt[:])
            nc.gpsimd.collective_compute(
                "AllGather",
                mybir.AluOpType.bypass,
                replica_groups=[list(range(N_CORES))],
                ins=[wcc_in[:].opt()],
                outs=[wcc_out[:].opt()],
            )
        cc_in = dram.tile([GRP, H], F32)
        ccL = dram.tile([N_PAIR, H], F32, addr_space=shared, name=f"{P_}ccL")
        cc2_in = dram.tile([N_PAIR, 1], F32)
        cc2_shape = [N_CORES * N_PAIR, 1] if final_ag else [N_PAIR, 1]
        cc2_out = dram.tile(cc2_shape, F32, addr_space=shared, name=f"{P_}cc2o")

        xin = ctx.enter_context(tc.tile_pool(name=f"{P_}xin", bufs=xin_bufs))
        psA = ctx.enter_context(tc.tile_pool(name=f"{P_}psA", bufs=2, space="PSUM"))
        # fastfin frees a bank for psZ (group 3 never pools into psA/psB, so
        # bufs=1 only stalls a group boundary by the ~0.1us copy drain)
        psB = ctx.enter_context(
            tc.tile_pool(name=f"{P_}psB", bufs=1 if fastfin else 2, space="PSUM")
        )
        psT = ctx.enter_context(tc.tile_pool(name=f"{P_}psT", bufs=2, space="PSUM"))
        psO = ctx.enter_context(tc.tile_pool(name=f"{P_}psO", bufs=1, space="PSUM"))
        psS = ctx.enter_context(tc.tile_pool(name=f"{P_}psS", bufs=1, space="PSUM"))
        psZ = (
            ctx.enter_context(tc.tile_pool(name=f"{P_}psZ", bufs=1, space="PSUM"))
            if fastfin
            else None
        )
        s2 = ctx.enter_context(tc.tile_pool(name=f"{P_}s2", bufs=1))
        s2t = ctx.enter_context(tc.tile_pool(name=f"{P_}s2t", bufs=2))

        pSLL = psS.tile([N_PAIR, N_PAIR], F32)  # S_LL bank
        # one full psZ bank: cols 0:48 = zzT3 (transposed pooled sums),
        # cols 64:128 = the S_OL matmul output (PSUM tiles are bank-granular)
        zzps = psZ.tile([128, 512], F32, name=f"{P_}zzps") if fastfin else None
        zzT3p = zzps

        def consume_L():
            """Normalize gathered left rows, transpose, S_LL, pair-sum."""
            zL = s2.tile([N_PAIR, H], F32, name=f"{P_}zL")
            # gathered row c*8+j holds logical batch c+8j -> partition 8j+c
            src = ccL.rearrange("(c j) e -> j c e", c=N_CORES)
            cdma(zL[:], src)
            sqs = s2t.tile([N_PAIR, H], F32, tag="sqL")
            nc.vector.tensor_mul(sqs[:], zL[:], zL[:])
            ssn = s2t.tile([N_PAIR, 1], F32, tag="ssL")
            nc.vector.reduce_sum(out=ssn[:], in_=sqs[:], axis=AX.X)
            nrm = s2t.tile([N_PAIR, 1], F32, tag="nrL")
            nc.scalar.activation(nrm[:], ssn[:], AF.Sqrt, scale=TAU)
            rn = s2t.tile([N_PAIR, 1], F32, tag="rnL")
            nc.vector.reciprocal(rn[:], nrm[:])
            nc.vector.tensor_scalar_mul(zL[:], zL[:], rn[:, 0:1])
            for k in range(6):
                pt = psT.tile([128, N_PAIR], F32, tag="pt")
                nc.tensor.transpose(
                    pt[:, 0:N_PAIR],
                    zL[:, k * 128 : (k + 1) * 128],
                    idt[0:N_PAIR, 0:N_PAIR],
                )
                ccopy(zLT[:, k * N_PAIR : (k + 1) * N_PAIR], pt[:, 0:N_PAIR])
            for k in range(6):
                nc.tensor.matmul(
                    pSLL[:],
                    lhsT=zLT[:, k * N_PAIR : (k + 1) * N_PAIR],
                    rhs=zLT[:, k * N_PAIR : (k + 1) * N_PAIR],
                    start=(k == 0),
                    stop=(k == 5),
                )
            mt2 = s2.tile([N_PAIR, N_PAIR], F32, name=f"{P_}mt2")
            nc.vector.tensor_mul(mt2[:], pSLL[:], triu_t[:])
            rs = s2.tile([N_PAIR, 1], F32, name=f"{P_}rs")
            nc.vector.reduce_sum(out=rs[:], in_=mt2[:], axis=AX.X)
            if final_ag:
                return rs  # column finish: no transpose needed
            # transpose to [1,64] so the whole finish chain is single-row
            prT = psT.tile([128, N_PAIR], F32, tag="pt")
            nc.tensor.transpose(prT[0:1, 0:N_PAIR], rs[:], idt[0:N_PAIR, 0:N_PAIR])
            rsT = s2.tile([1, N_PAIR], F32, name=f"{P_}rsT")
            ccopy(rsT[:], prT[0:1, 0:N_PAIR])
            return rsT

        def group_consume(g):
            """Normalize own group rows, transpose, S_LO columns for group g."""
            rows = zzg[g][:]
            sq8 = s2t.tile([GRP, H], F32, tag="sq8")
            nc.vector.tensor_mul(sq8[:], rows, rows)
            sn8 = s2t.tile([GRP, 1], F32, tag="sn8")
            nc.vector.reduce_sum(out=sn8[:], in_=sq8[:], axis=AX.X)
            nr8 = s2t.tile([GRP, 1], F32, tag="nr8")
            nc.scalar.activation(nr8[:], sn8[:], AF.Sqrt, scale=TAU)
            rn8 = s2t.tile([GRP, 1], F32, tag="rn8")
            nc.vector.reciprocal(rn8[:], nr8[:])
            nc.vector.tensor_scalar_mul(rows, rows, rn8[:, 0:1])
            for k in range(6):
                pt = psT.tile([128, N_PAIR], F32, tag="pt")
                nc.tensor.transpose(
                    pt[:, 0:GRP], rows[:, k * 128 : (k + 1) * 128], idt[0:GRP, 0:GRP]
                )
                ccopy(
                    zzT[:, k * B_SH + g * GRP : k * B_SH + (g + 1) * GRP],
                    pt[:, 0:GRP],
                )
            pO = psO.tile([N_PAIR, GRP], F32, tag="pO")
            for k in range(6):
                nc.tensor.matmul(
                    pO[:],
                    lhsT=zLT[:, k * N_PAIR : (k + 1) * N_PAIR],
                    rhs=zzT[:, k * B_SH + g * GRP : k * B_SH + (g + 1) * GRP],
                    start=(k == 0),
                    stop=(k == 5),
                )
            if g < NG - 1:
                ccopy(sLO[:, g * GRP : (g + 1) * GRP], pO[:])
            return pO

        # ---- stage 1: stream + grouped pooling ------------------------------
        if xlayout == "pc":
            x4 = x.rearrange("b (p c) e -> b p c e", c=4)
        else:
            x4 = x.rearrange("b (c p) e -> b p c e", p=128)
        rsT = None
        se_a = [None]
        se_ar = [None]
        pA = pB = None
        for b in range(B_SH):
            g, jg = divmod(b, GRP)
            if "s1" in stages:
                # two half-tile DMAs on the last batch so the chunk-folding
                # adds start when the first half lands (shaves ~2us off the
                # tail); earlier batches use stream_split (bigger DMAs have
                # better HBM efficiency)
                # mix: scalar ring only where ACT compute is guaranteed quiet
                # (consume_L/group_consume land at b=15-17 and 27, exp at 27)
                MIXSB = {1, 3, 5, 7, 9, 11, 13, 21, 23, 25, 29}
                if dma_eng == "alt" and b % 2:
                    eng = nc.scalar
                elif dma_eng == "altg" and b % 2:
                    eng = nc.gpsimd
                elif dma_eng == "mix" and b in MIXSB:
                    eng = nc.scalar
                else:
                    eng = nc.sync
                xt = xin.tile([128, 4 * H], F32)
                if b == B_SH - 1:
                    nsp = 4 if fastfin else 2
                else:
                    nsp = stream_split
                w = 4 * H // nsp
                cw = 4 // nsp
                for s in range(nsp):
                    e_s = eng
                    if dma_eng == "mix" and b == B_SH - 1:
                        # last batch: one half per ring for earliest landing
                        e_s = nc.sync if s == 0 else nc.scalar
                    e_s.dma_start(
                        xt[:, s * w : (s + 1) * w], x4[b, :, s * cw : (s + 1) * cw]
                    )
            if "pool" in stages and fastfin and g == NG - 1:
                # group 3: transposed pooling.  Column 8k+jg of the psZ bank
                # gets pooledT chunk k of this batch via a [128x128]-stationary
                # ones-column matmul, so the tail needs no PSUM->SBUF row copy,
                # no normalize-rows pass, and no PE transposes.
                if jg == 0:
                    # clear the bank once (start=True covers all 48 cols)
                    nc.tensor.matmul(
                        zzT3p[:, 0 : 6 * GRP],
                        lhsT=idt[:],
                        rhs=zeros48[:],
                        start=True,
                        stop=False,
                    )
                if b == B_SH - 1:
                    # quarter-DMAs land progressively: accumulate chunks as
                    # they arrive so only ONE add remains after the last byte
                    nc.vector.tensor_add(xt[:, 0:H], xt[:, 0:H], xt[:, H : 2 * H])
                    nc.vector.tensor_add(
                        xt[:, 0:H], xt[:, 0:H], xt[:, 2 * H : 3 * H]
                    )
                    nc.vector.tensor_add(
                        xt[:, 0:H], xt[:, 0:H], xt[:, 3 * H : 4 * H]
                    )
                else:
                    nc.vector.tensor_add(xt[:, 0:H], xt[:, 0:H], xt[:, H : 2 * H])
                    nc.vector.tensor_add(
                        xt[:, 2 * H : 3 * H],
                        xt[:, 2 * H : 3 * H],
                        xt[:, 3 * H : 4 * H],
                    )
                    nc.vector.tensor_add(
                        xt[:, 0:H], xt[:, 0:H], xt[:, 2 * H : 3 * H]
                    )
                for k in range(6):
                    nc.tensor.matmul(
                        zzT3p[:, 8 * k + jg : 8 * k + jg + 1],
                        lhsT=xt[:, k * 128 : (k + 1) * 128],
                        rhs=onesc[:, 0:1],
                        start=False,
                        stop=True,
                    )
            elif "pool" in stages:
                if jg == 0:
                    pA = psA.tile([GRP, 512], F32, tag="A")
                    pB = psB.tile([GRP, 256], F32, tag="B")
                lw = e8[:, GRP * jg : GRP * (jg + 1)]
                if b == B_SH - 1:
                    # last batch: fold only half 1 on DVE; chunks 2 and 3 go
                    # straight into the (warm) PE accumulation.  Shorter tail
                    # path than add2 -> add3 -> matmul, and drops the DVE->PE
                    # handoff from the critical path.  Emission order: chunks
                    # 2,3 first (only need the half-2 DMA), folded chunk last.
                    nc.vector.tensor_add(xt[:, 0:H], xt[:, 0:H], xt[:, H : 2 * H])
                    for base in (2 * H, 3 * H, 0):
                        nc.tensor.matmul(
                            pA[:, :],
                            lhsT=lw,
                            rhs=xt[:, base : base + 512],
                            start=False,
                            stop=(base == 0),
                        )
                    for base in (2 * H, 3 * H, 0):
                        nc.tensor.matmul(
                            pB[:, :],
                            lhsT=lw,
                            rhs=xt[:, base + 512 : base + H],
                            start=False,
                            stop=(base == 0),
                        )
                else:
                    # fold the 4 sequence chunks on the (otherwise idle) DVE
                    # so PE streams 768 cols/batch instead of 3072 (PE at
                    # cold 1.2GHz was the stream bottleneck at 8 mm/batch)
                    nc.vector.tensor_add(xt[:, 0:H], xt[:, 0:H], xt[:, H : 2 * H])
                    nc.vector.tensor_add(
                        xt[:, 2 * H : 3 * H],
                        xt[:, 2 * H : 3 * H],
                        xt[:, 3 * H : 4 * H],
                    )
                    nc.vector.tensor_add(xt[:, 0:H], xt[:, 0:H], xt[:, 2 * H : 3 * H])
                    nc.tensor.matmul(
                        pA[:, :],
                        lhsT=lw,
                        rhs=xt[:, 0:512],
                        start=(jg == 0),
                        stop=(jg == GRP - 1),
                    )
                    nc.tensor.matmul(
                        pB[:, :],
                        lhsT=lw,
                        rhs=xt[:, 512:H],
                        start=(jg == 0),
                        stop=(jg == GRP - 1),
                    )
                if jg == GRP - 1:
                    # split across ACT and DVE so the two bank copies run in
                    # parallel (group 3's copies sit on the exposed tail);
                    # act_light puts both on DVE to keep the ACT ring clean
                    if act_light:
                        nc.vector.tensor_copy(zzg[g][:, 0:512], pA[:])
                    else:
                        nc.scalar.copy(zzg[g][:, 0:512], pA[:])
                    nc.vector.tensor_copy(zzg[g][:, 512:H], pB[:])
            if "cc" in stages:
                if b == GRP - 1:
                    cdma(cc_in[:], zzg[0][:])
                    if use_collective:
                        nc.gpsimd.collective_compute(
                            "AllGather",
                            mybir.AluOpType.bypass,
                            replica_groups=[list(range(N_CORES))],
                            ins=[cc_in[:].opt()],
                            outs=[ccL[:].opt()],
                        )
                    else:
                        for c in range(N_CORES):
                            nc.scalar.dma_start(
                                ccL[c * GRP : (c + 1) * GRP, :], cc_in[:]
                            )
                if "s2" in stages:
                    if b == 15:
                        rsT = consume_L()
                    elif b == 16:
                        group_consume(0)
                    elif b == 17:
                        group_consume(1)
                    elif b == 27:
                        group_consume(2)
                        # exp for groups 0..2 hidden under the stream
                        eta = s2.tile([N_PAIR, 3 * GRP], F32, name=f"{P_}eta")
                        sa = s2.tile([N_PAIR, 1], F32, name=f"{P_}sea")
                        nc.scalar.activation(
                            eta[:],
                            sLO[:, 0 : 3 * GRP],
                            AF.Exp,
                            scale=1.0,
                            accum_out=sa[:],
                        )
                        se_a[0] = sa
                        if fastfin:
                            # row form for the 1-descriptor collective-input
                            # DMA at the tail (hidden here under the stream)
                            saT = psT.tile([128, N_PAIR], F32, tag="pt")
                            nc.tensor.transpose(
                                saT[0:1, 0:N_PAIR],
                                sa[:],
                                idt[0:N_PAIR, 0:N_PAIR],
                            )
                            sa_row = s2.tile([1, N_PAIR], F32, name=f"{P_}sarow")
                            ccopy(sa_row[:], saT[0:1, 0:N_PAIR])
                            se_ar[0] = sa_row
            if "pool" in stages and "s2" in stages and warm and b == B_SH - 2:
                # ~3.5us of back-to-back dummy PE work, hidden under batch
                # 31's DMA window: trips the HAM activity monitor (4096-cycle
                # window) so the tail's matmuls/transposes run at the warm
                # 2.4GHz clock instead of the cold 1.2GHz default
                for _ in range(30):
                    ptw = psT.tile([128, N_PAIR], F32, tag="pt")
                    nc.tensor.transpose(
                        ptw[:, 0:N_PAIR],
                        idt[0:N_PAIR, :],
                        idt[0:N_PAIR, 0:N_PAIR],
                    )

        if "cc" not in stages or "s2" not in stages:
            return
        if skip_finish:
            fin_cut = "pre"
        if fastfin:
            if fin_cut == "pre":
                return
            # ---- fastfin tail: zzT3 (PSUM, transposed raw sums) -> S_OL ----
            # one [128,48] copy replaces the row copy + normalize-rows +
            # 6 transposes of group_consume; norms via ones-column matmuls.
            zzT3sb = s2.tile([128, 6 * GRP], F32, name=f"{P_}zzT3sb")
            nc.scalar.copy(zzT3sb[:], zzT3p[:, 0 : 6 * GRP])
            # S_OL[j, i] = zraw_own_j . zhat_L_i (normalize by rn8f after);
            # emitted FIRST so PE's long pole starts as soon as the copy
            # lands.  Output shares the psZ bank; its start=True clear of
            # zzT3 is safe (zzT3's only reader, the zzT3sb copy, is upstream
            # of these matmuls).
            pOL = zzps[0:GRP, 64 : 64 + N_PAIR]
            for k in range(6):
                nc.tensor.matmul(
                    pOL,
                    lhsT=zzT3sb[:, 8 * k : 8 * (k + 1)],
                    rhs=zLT[:, k * N_PAIR : (k + 1) * N_PAIR],
                    start=(k == 0),
                    stop=(k == 5),
                )
            sq48 = s2t.tile([128, 6 * GRP], F32, tag="sq48")
            nc.vector.tensor_mul(sq48[:], zzT3sb[:], zzT3sb[:])
            pn = psT.tile([128, N_PAIR], F32, tag="pt")
            for k in range(6):
                nc.tensor.matmul(
                    pn[0:GRP, 0:1],
                    lhsT=sq48[:, 8 * k : 8 * (k + 1)],
                    rhs=onesc[:, 0:1],
                    start=(k == 0),
                    stop=(k == 5),
                )
            nr8f = s2t.tile([GRP, 1], F32, tag="nr8f")
            nc.scalar.activation(nr8f[:], pn[0:GRP, 0:1], AF.Sqrt, scale=TAU)
            rn8f = s2t.tile([GRP, 1], F32, tag="rn8f")
            nc.vector.reciprocal(rn8f[:], nr8f[:])
            sOL = s2.tile([GRP, N_PAIR], F32, name=f"{P_}sOL")
            nc.vector.tensor_scalar_mul(sOL[:], pOL, rn8f[:, 0:1])
            eOL = s2.tile([GRP, N_PAIR], F32, name=f"{P_}eOL")
            nc.scalar.activation(eOL[:], sOL[:], AF.Exp, scale=1.0)
            # D3 straight to row form: with the (hidden) se_a row, the
            # collective-input DMA is a single 256B descriptor
            pd3r = psT.tile([128, N_PAIR], F32, tag="pt")
            nc.tensor.matmul(
                pd3r[0:1, 0:N_PAIR],
                lhsT=onesc[0:GRP, 0:1],
                rhs=eOL[:],
                start=True,
                stop=True,
            )
            se_row = s2.tile([1, N_PAIR], F32, name=f"{P_}serow")
            nc.vector.tensor_add(se_row[:], se_ar[0], pd3r[0:1, 0:N_PAIR])
        else:
            pO3 = group_consume(3)
            if fin_cut == "pre":
                return

            # ---- finish: partial denominators -> AllReduce -> loss ----------
            # exp over group-3 columns straight from PSUM; groups 0..2 were
            # exp'd mid-stream (exp_a).  se = se_a + se_b, diag removed later.
            et = s2.tile([N_PAIR, GRP], F32, name=f"{P_}et")
            se_b = s2.tile([N_PAIR, 1], F32, name=f"{P_}seb")
            nc.scalar.activation(et[:], pO3[:], AF.Exp, scale=1.0, accum_out=se_b[:])
            se = s2.tile([N_PAIR, 1], F32, name=f"{P_}se")
            nc.vector.tensor_add(se[:], se_a[0], se_b[:])
        if fin_cut == "se":
            return
        if fastfin:
            nc.scalar.dma_start(cc2_in[:].rearrange("a b -> b a"), se_row[:])
        else:
            nc.scalar.dma_start(cc2_in[:], se[:])
        if fin_cut == "dma":
            return
        if use_collective:
            nc.gpsimd.collective_compute(
                "AllGather" if final_ag else "AllReduce",
                mybir.AluOpType.bypass if final_ag else mybir.AluOpType.add,
                replica_groups=[list(range(N_CORES))],
                ins=[cc2_in[:].opt()],
                outs=[cc2_out[:].opt()],
            )
        elif final_ag:
            for c in range(N_CORES):
                nc.scalar.dma_start(
                    cc2_out[c * N_PAIR : (c + 1) * N_PAIR, :], cc2_in[:]
                )
        else:
            nc.scalar.dma_start(cc2_out[:], cc2_in[:])
        if fin_cut == "cc":
            return
        if final_ag:
            # column finish: partition c <- core c's [64] (8 contiguous
            # descriptors), PE ones-matmul sums the cores, then the whole
            # chain stays [64,1] (no transposes; rsT here is the rs column).
            l8 = s2.tile([N_CORES, N_PAIR], F32, name=f"{P_}l8")
            nc.scalar.dma_start(
                l8[:], cc2_out[:].rearrange("(c i) o -> c (i o)", c=N_CORES)
            )
            pD = psT.tile([128, N_PAIR], F32, tag="pt")
            nc.tensor.matmul(
                pD[0:N_PAIR, 0:1],
                lhsT=l8[:],
                rhs=onesc[0:N_CORES, 0:1],
                start=True,
                stop=True,
            )
            ldc = s2.tile([N_PAIR, 1], F32, name=f"{P_}ldc")
            nc.scalar.activation(ldc[:], pD[0:N_PAIR, 0:1], AF.Ln, bias=negd64[:])
            t1c = s2.tile([N_PAIR, 1], F32, name=f"{P_}t1c")
            nc.vector.tensor_mul(t1c[:], ldc[:], cnt_col[:])
            prc = s2.tile([N_PAIR, 1], F32, name=f"{P_}prc")
            nc.vector.tensor_sub(prc[:], t1c[:], rsT[:])
            pres = psT.tile([128, N_PAIR], F32, tag="pt")
            nc.tensor.matmul(
                pres[0:1, 0:1],
                lhsT=prc[:],
                rhs=onesc[0:N_PAIR, 0:1],
                start=True,
                stop=True,
            )
            res = s2.tile([1, 1], F32, name=f"{P_}res")
            nc.scalar.copy(res[:], pres[0:1, 0:1])
            nc.sync.dma_start(out[0:1, 0:1], res[:])
            return
        # single-row finish: read D back as [1,64], remove the diagonal
        # (each row owned by exactly one core -> sum of e^{1/tau} once)
        # as a constant Ln bias, then weighted-accumulate to the scalar.
        lds = s2.tile([1, N_PAIR], F32, name=f"{P_}lds")
        nc.scalar.dma_start(lds[:], cc2_out[:].rearrange("a b -> b a"))
        ld = s2.tile([1, N_PAIR], F32, name=f"{P_}ld")
        nc.scalar.activation(ld[:], lds[:], AF.Ln, bias=negd[:])
        t1 = s2.tile([1, N_PAIR], F32, name=f"{P_}t1")
        nc.vector.tensor_mul(t1[:], ld[:], cnt_t[:])
        pr = s2.tile([1, N_PAIR], F32, name=f"{P_}pr")
        nc.vector.tensor_sub(pr[:], t1[:], rsT[:])
        junk = s2.tile([1, N_PAIR], F32, name=f"{P_}junk")
        res = s2.tile([1, 1], F32, name=f"{P_}res")
        nc.scalar.activation(junk[:], pr[:], AF.Copy, accum_out=res[:])
        nc.scalar.dma_start(out[0:1, 0:1], res[:])


KERNEL_VERSION = "v2"


def build_nc(reps=1, version=None, serialize_reps=False, **body_kwargs):
    version = version or KERNEL_VERSION
    nc = bacc.Bacc("TRN2", target_bir_lowering=False, debug=False, num_devices=N_CORES)
    x = nc.dram_tensor("x", [B_SH, S, H], F32, kind="ExternalInput")
    ident = nc.dram_tensor("ident", [128, 128], F32, kind="ExternalInput")
    triu = nc.dram_tensor("triu", [N_PAIR, N_PAIR], F32, kind="ExternalInput")
    if version == "v1":
        cnt = nc.dram_tensor("cnt", [N_PAIR, 1], F32, kind="ExternalInput")
        dmask = nc.dram_tensor("dmask", [N_PAIR, B], F32, kind="ExternalInput")
    else:
        cnt = nc.dram_tensor("cnt", [1, N_PAIR], F32, kind="ExternalInput")
    out = nc.dram_tensor("loss", [1, 1], F32, kind="ExternalOutput")
    with tile.TileContext(nc) as tc:
        for r in range(reps):
            prefix = f"r{r}_" if reps > 1 else ""
            if serialize_reps and r > 0:
                # block this rep's stream-issue rings on the previous rep's
                # final out-write so K-diff measures honest serial per-rep
                # time (no cross-rep overlap games)
                with tc.tile_pool(name=f"ser{r}", bufs=1) as serp:
                    tok = serp.tile([1, 2], F32, name=f"tok{r}")
                    nc.sync.dma_start(tok[0:1, 0:1], out.ap()[0:1, 0:1])
                    nc.scalar.dma_start(tok[0:1, 1:2], out.ap()[0:1, 0:1])
            if version == "v1":
                _body(
                    tc,
                    x.ap(),
                    ident.ap(),
                    dmask.ap(),
                    triu.ap(),
                    cnt.ap(),
                    out.ap(),
                    prefix=prefix,
                    **body_kwargs,
                )
            else:
                _body_v2(
                    tc,
                    x.ap(),
                    ident.ap(),
                    triu.ap(),
                    cnt.ap(),
                    out.ap(),
                    prefix=prefix,
                    **body_kwargs,
                )
    nc.compile()
    return nc


def const_inputs(version=None):
    version = version or KERNEL_VERSION
    ident = np.eye(128, dtype=np.float32)
    triu = np.triu(np.ones((N_PAIR, N_PAIR), dtype=np.float32), k=1)
    cnt = (N_PAIR - 1 - np.arange(N_PAIR, dtype=np.float32)).reshape(N_PAIR, 1)
    if version == "v1":
        dmask = np.zeros((N_PAIR, B), dtype=np.float32)
        dmask[np.arange(N_PAIR), np.arange(N_PAIR)] = NEG
        return {"ident": ident, "triu": triu, "cnt": cnt, "dmask": dmask}
    # v2: fold the final -2/n*(n-1) scale into triu and cnt so the loss is
    # a plain accumulate after the AllReduce
    sc = -2.0 / N_PAIR * (N_PAIR - 1)
    return {
        "ident": ident,
        "triu": (sc * triu).astype(np.float32),
        "cnt": (sc * cnt).astype(np.float32).reshape(1, N_PAIR),
    }


def make_in_maps(last_hidden_states, input_mask, version=None):
    version = version or KERNEL_VERSION
    del input_mask  # cancels exactly in the L2 normalization (see half_tail)
    x = np.asarray(last_hidden_states, dtype=np.float32)
    consts = const_inputs(version)
    return [
        {"x": np.ascontiguousarray(x[c::N_CORES]), **consts}  # logical c+8j
        for c in range(N_CORES)
    ]


_CACHE = {}


def get_nc(version=None):
    key = version or KERNEL_VERSION
    if key not in _CACHE:
        _CACHE[key] = build_nc(version=key)
    return _CACHE[key]


def kernel(last_hidden_states, input_mask):
    nc = get_nc()
    in_maps = make_in_maps(last_hidden_states, input_mask)
    res = bass_utils.run_bass_kernel_spmd(nc, in_maps, core_ids=list(range(N_CORES)))
    return np.asarray(res.results[0]["loss"], dtype=np.float32).reshape(())

